# revision 7
# baseline (speedup 1.0000x reference)
"""Trainium2 Bass kernel for nn_BiLSTM_5970004542177.

Model: 2-layer bidirectional LSTM (Keras gate order i,f,g,o), B=128, T=256,
D=U=256, residual on layer 1, merge_mode='ave'.

Device mapping (8 NeuronCores, SPMD single program, no cross-core comm):
  core = (direction, batch quarter): cores 0-3 forward, 4-7 backward
  (backward = time-reversed input, host un-reverses the output).

Each core runs BOTH layers of its chain at B=32 in transposed layout
(partitions = units, free = batch), chunk-interleaved: layer-1 chunk j-1 is
emitted right after layer-0 chunk j, so the two recurrences' serial
dependency chains overlap across engines.  The input projection W^T x + b
is fused into the same PSUM accumulation group as the per-step recurrence
matmuls (bias rides a third K-tile against a constant ones-row).  Layer 1
reads layer 0's h history directly from SBUF and emits
out = 0.5*(h1 + h0); the host adds fw+bw shards and restores (B, T, U).
"""
import sys

if "/opt/trn_rl_repo" not in sys.path:
    sys.path.insert(0, "/opt/trn_rl_repo")

import numpy as np
import ml_dtypes

B = 32            # per-core batch (128 / 4 quarters)
T = 256
D = 256
U = 256
C = 32            # chunk length (steps)
NC = T // C
GS = 4            # steps per PSUM group
NKW = 3           # proj K-tiles (2 data + bias row)
NKR = 2
NM = 8
CB = C * B

_CACHE = {}


class _Unit:
    """Emission helper for one LSTM layer; supports fine interleaving."""

    def __init__(self, nc, mybir, pools, tag, W_sb, R_sb, rhs_fn, hist_ap,
                 h_prev0, c_sb):
        self.nc, self.mybir, self.pools = nc, mybir, pools
        self.tag = tag
        self.W_sb, self.R_sb = W_sb, R_sb
        self.rhs_fn, self.hist_ap = rhs_fn, hist_ap
        self.h_prev0, self.c_sb = h_prev0, c_sb
        self.zp = None

    def _proj_mms(self, zp, g, m_lo, m_hi):
        nc = self.nc
        for m in range(m_lo, m_hi):
            for k in range(NKW):
                nc.tensor.matmul(
                    zp[:, m, :],
                    self.W_sb[:, (m * NKW + k) * 128:(m * NKW + k + 1) * 128],
                    self.rhs_fn(k, g),
                    start=(k == 0 and (m * GS * B) % 512 == 0), stop=False,
                    skip_group_check=True,
                )

    def _new_zp(self):
        F32 = self.mybir.dt.float32
        zp_t = self.pools["psum"].tile([128, NM, GS * B], F32,
                                       tag="zp" + self.tag)
        return zp_t

    def emit_proj(self, g):
        self.zp = self._new_zp()
        self._proj_mms(self.zp, g, 0, NM)

    def emit_proj_slice(self, g, sl):
        """Emit a quarter of group g's projection (2 M-strips); used to fill
        PE stalls during the previous group's recurrence steps."""
        if sl == 0:
            self.zp_next = self._new_zp()
        self._proj_mms(self.zp_next, g, 2 * sl, 2 * sl + 2)

    def advance_group(self):
        self.zp = self.zp_next

    def emit_rec_mms(self, g, sl):
        nc = self.nc
        s = g * GS + sl
        h_prev = self.h_prev0 if s == 0 else self.hist_ap[:, s - 1]
        for m in range(NM):
            for k in range(NKR):
                nc.tensor.matmul(
                    self.zp[:, m, sl * B:(sl + 1) * B],
                    self.R_sb[:, (m * NKR + k) * 128:(m * NKR + k + 1) * 128],
                    h_prev[:, k, :],
                    start=False, stop=(k == NKR - 1),
                    skip_group_check=True,
                )

    def emit_sigmoid(self, g, sl):
        nc, mybir = self.nc, self.mybir
        BF16 = mybir.dt.bfloat16
        SIG = mybir.ActivationFunctionType.Sigmoid
        work = self.pools["work"]
        self.gt = work.tile([128, NM, B], BF16, tag="gt" + self.tag)
        zs = self.zp[:, :, sl * B:(sl + 1) * B]
        # all four gates through one sigmoid; the g columns were pre-scaled
        # by 2 on the host so tanh(zg) = 2*sigmoid(2 zg) - 1 = 2*gt_g - 1
        nc.scalar.activation(self.gt[:], zs[:], SIG)

    def emit_t1_cmul(self):
        nc, mybir = self.nc, self.mybir
        BF16 = mybir.dt.bfloat16
        MULT = mybir.AluOpType.mult
        work = self.pools["work"]
        gt = self.gt
        self.t1 = work.tile([128, 2, B], BF16, tag="t1" + self.tag)
        # c = f*c + i*(2*sg - 1) = f*c + (2*(i*sg) - i)
        # t1 = i*sg as STT to hit the 4x DVE perf mode (all-bf16, SBUF)
        nc.vector.scalar_tensor_tensor(self.t1[:], gt[:, 0:2, :], 1.0,
                                       gt[:, 4:6, :], op0=MULT, op1=MULT)
        # c *= f on the otherwise-idle GpSimd engine, overlaps t1/t2
        nc.gpsimd.tensor_tensor(self.c_sb[:], self.c_sb[:], gt[:, 2:4, :],
                                op=MULT)

    def emit_t2(self):
        nc, mybir = self.nc, self.mybir
        BF16 = mybir.dt.bfloat16
        MULT = mybir.AluOpType.mult
        SUB = mybir.AluOpType.subtract
        work = self.pools["work"]
        self.t2 = work.tile([128, 2, B], BF16, tag="t2" + self.tag)
        nc.vector.scalar_tensor_tensor(self.t2[:], self.t1[:], 2.0,
                                       self.gt[:, 0:2, :], op0=MULT, op1=SUB)

    def emit_cadd(self):
        nc, mybir = self.nc, self.mybir
        ADD = mybir.AluOpType.add
        nc.vector.tensor_tensor(self.c_sb[:], self.c_sb[:], self.t2[:],
                                op=ADD)

    def emit_tanh(self):
        nc, mybir = self.nc, self.mybir
        BF16 = mybir.dt.bfloat16
        TANH = mybir.ActivationFunctionType.Tanh
        work = self.pools["work"]
        self.tct = work.tile([128, 2, B], BF16, tag="tc" + self.tag)
        nc.scalar.activation(self.tct[:], self.c_sb[:], TANH)

    def emit_ho(self, g, sl):
        nc, mybir = self.nc, self.mybir
        MULT = mybir.AluOpType.mult
        s = g * GS + sl
        nc.vector.scalar_tensor_tensor(self.hist_ap[:, s], self.gt[:, 6:8, :],
                                       1.0, self.tct[:], op0=MULT, op1=MULT)


def _build():
    import concourse.bacc as bacc
    import concourse.tile as tile
    from concourse import mybir

    F32 = mybir.dt.float32
    BF16 = mybir.dt.bfloat16
    ADD = mybir.AluOpType.add

    nc = bacc.Bacc("TRN2", target_bir_lowering=False, debug=False)
    W0d = nc.dram_tensor("Wp0", [128, NKW * NM * 128], BF16,
                         kind="ExternalInput")
    R0d = nc.dram_tensor("Rp0", [128, NKR * NM * 128], BF16,
                         kind="ExternalInput")
    W1d = nc.dram_tensor("Wp1", [128, NKW * NM * 128], BF16,
                         kind="ExternalInput")
    R1d = nc.dram_tensor("Rp1", [128, NKR * NM * 128], BF16,
                         kind="ExternalInput")
    Xd = nc.dram_tensor("Xp", [128, 2, T * B], BF16, kind="ExternalInput")
    OutD = nc.dram_tensor("Out", [128, 2, T * 2 * B], BF16,
                          kind="ExternalOutput")

    with tile.TileContext(nc) as tc:
        with (
            tc.tile_pool(name="const", bufs=1) as const,
            tc.tile_pool(name="state", bufs=1) as state,
            tc.tile_pool(name="work", bufs=6) as work,
            tc.tile_pool(name="psum", bufs=2, space="PSUM") as psum,
        ):
            W0 = const.tile([128, NKW * NM * 128], BF16)
            R0 = const.tile([128, NKR * NM * 128], BF16)
            W1 = const.tile([128, NKW * NM * 128], BF16)
            R1 = const.tile([128, NKR * NM * 128], BF16)
            nc.sync.dma_start(out=W0[:], in_=W0d[:])
            nc.sync.dma_start(out=R0[:], in_=R0d[:])
            nc.sync.dma_start(out=W1[:], in_=W1d[:])
            nc.sync.dma_start(out=R1[:], in_=R1d[:])

            xin = const.tile([128, 2, T * B], BF16)
            # per-chunk slices so chunk 0's matmuls start after 1/NC of the
            # input transfer instead of the whole 4 MB
            for jj in range(NC):
                nc.sync.dma_start(out=xin[:, :, jj * CB:(jj + 1) * CB],
                                  in_=Xd[:, :, jj * CB:(jj + 1) * CB])
            ones = const.tile([128, GS * B], BF16)
            nc.vector.memset(ones[:], 0.0)
            nc.vector.memset(ones[0:1, :], 1.0)

            hist0 = state.tile([128, T, 2, B], BF16)
            hist1 = state.tile([128, T, 2, B], BF16)
            h00 = state.tile([128, 2, B], BF16)
            c0 = state.tile([128, 2, B], F32)
            c1 = state.tile([128, 2, B], F32)
            nc.vector.memset(h00[:], 0.0)
            nc.vector.memset(c0[:], 0.0)
            nc.vector.memset(c1[:], 0.0)

            pools = {"psum": psum, "work": work}

            def rhs_l0(j):
                def fn(k, g):
                    if k < 2:
                        a = j * C + g * GS
                        return xin[:, k, a * B:(a + GS) * B]
                    return ones[:]
                return fn

            def rhs_l1(j):
                def fn(k, g):
                    if k < 2:
                        a = j * C + g * GS
                        return hist0[:, a:a + GS, k, :]
                    return ones[:]
                return fn

            NG = C // GS
            for j in range(NC + 1):
                u0 = u1 = None
                if j < NC:
                    u0 = _Unit(nc, mybir, pools, "a", W0, R0, rhs_l0(j),
                               hist0[:, j * C:(j + 1) * C],
                               h00 if j == 0 else hist0[:, j * C - 1], c0)
                if j >= 1:
                    i = j - 1
                    u1 = _Unit(nc, mybir, pools, "b", W1, R1, rhs_l1(i),
                               hist1[:, i * C:(i + 1) * C],
                               h00 if i == 0 else hist1[:, i * C - 1], c1)
                # op-level interleaved emission so every engine's queue
                # alternates between the two units' chains in readiness
                # order; the next group's projection matmuls are sliced
                # between steps to keep TensorE fed during recurrence stalls.
                units = [u for u in (u0, u1) if u is not None]
                for g in range(NG):
                    for u in units:
                        if g == 0:
                            u.emit_proj(0)
                        else:
                            u.advance_group()
                    for sl in range(GS):
                        for u in units:
                            u.emit_rec_mms(g, sl)
                        for u in units:
                            u.emit_sigmoid(g, sl)
                        for u in units:
                            u.emit_t1_cmul()
                        for u in units:
                            u.emit_t2()
                        for u in units:
                            u.emit_cadd()
                        for u in units:
                            u.emit_tanh()
                        for u in units:
                            u.emit_ho(g, sl)
                        if g + 1 < NG:
                            for u in units:
                                u.emit_proj_slice(g + 1, sl)
                # ship both layers' raw h chunks; the host does h0+h1
                if u0 is not None:
                    nc.sync.dma_start(
                        out=OutD[:, 0, j * C * 2 * B:(j + 1) * C * 2 * B],
                        in_=hist0[:, j * C:(j + 1) * C].rearrange(
                            "p c k b -> p (c k b)"))
                if u1 is not None:
                    i = j - 1
                    nc.sync.dma_start(
                        out=OutD[:, 1, i * C * 2 * B:(i + 1) * C * 2 * B],
                        in_=hist1[:, i * C:(i + 1) * C].rearrange(
                            "p c k b -> p (c k b)"))

    nc.compile()
    return nc


# ------------------------------------------------------------- host packing
def _pack_W_aug(W, b):
    out = np.zeros((128, NKW * NM * 128), np.float32)
    for m in range(NM):
        for k in range(NKW):
            col = (m * NKW + k) * 128
            if k < 2:
                out[:, col:col + 128] = W[k * 128:(k + 1) * 128,
                                          m * 128:(m + 1) * 128]
            else:
                out[0, col:col + 128] = b[m * 128:(m + 1) * 128]
    return out.astype(ml_dtypes.bfloat16)


def _pack_R(R):
    out = np.zeros((128, NKR * NM * 128), np.float32)
    for m in range(NM):
        for k in range(NKR):
            col = (m * NKR + k) * 128
            out[:, col:col + 128] = R[k * 128:(k + 1) * 128,
                                      m * 128:(m + 1) * 128]
    return out.astype(ml_dtypes.bfloat16)


def _pack_x(xs):
    """xs (B, T, D) -> [128, 2, T*B] bf16 (k-tile, t-major cols)."""
    xt = np.ascontiguousarray(np.transpose(xs, (2, 1, 0))).reshape(D, T * B)
    out = np.empty((128, 2, T * B), np.float32)
    out[:, 0, :] = xt[0:128]
    out[:, 1, :] = xt[128:256]
    return out.astype(ml_dtypes.bfloat16)


def _make_in_maps(x, kernels_fw, rec_fw, bias_fw, kernels_bw, rec_bw, bias_bw):
    x = np.asarray(x, np.float32)
    xr = x[:, ::-1, :]
    def g2(a):
        a = np.array(a, np.float32)
        a[..., 2 * U:3 * U] *= 2.0
        return a

    packs = {}
    for d, Ws, Rs, bs in (("fw", kernels_fw, rec_fw, bias_fw),
                          ("bw", kernels_bw, rec_bw, bias_bw)):
        packs[d] = [
            (_pack_W_aug(g2(Ws[li]), g2(bs[li])), _pack_R(g2(Rs[li])))
            for li in range(2)
        ]
    in_maps = []
    for core in range(8):
        d = "fw" if core < 4 else "bw"
        q = core % 4
        xs = (x if d == "fw" else xr)[q * B:(q + 1) * B]
        (W0, R0), (W1, R1) = packs[d]
        in_maps.append({"Wp0": W0, "Rp0": R0, "Wp1": W1, "Rp1": R1,
                        "Xp": _pack_x(xs)})
    return in_maps


def _unshard(results):
    full = np.zeros((128, T, U), np.float32)
    for core in range(8):
        d_rev = core >= 4
        q = core % 4
        raw = np.asarray(results[core]["Out"], dtype=np.float32)
        o = (raw[:, 0, :] + raw[:, 1, :]).reshape(128, T, 2, B)
        o = np.transpose(o, (3, 1, 2, 0)).reshape(B, T, U)
        if d_rev:
            o = o[:, ::-1, :]
        full[q * B:(q + 1) * B] += o
    full *= 0.5
    return full


def _setup_axon_profile_hook():
    try:
        import types
        import antenv
        mod = sys.modules.get("antenv.axon_hooks")
        if mod is None:
            mod = types.ModuleType("antenv.axon_hooks")
            holder = {"hook": None}
            mod.set_axon_ntff_profile_hook = lambda h: holder.update(hook=h)
            mod.get_axon_ntff_profile_hook = lambda: holder["hook"]
            sys.modules["antenv.axon_hooks"] = mod
            antenv.axon_hooks = mod
        from trn_agent_boot.trn_boot import _ntff_profile_via_ctypes
        hook = _ntff_profile_via_ctypes("/opt/axon/libaxon_pjrt.so")
        if hook is not None:
            mod.set_axon_ntff_profile_hook(hook)
        import concourse.bass_utils as bass_utils
        bass_utils.upload_artifacts = lambda tmpdir: tmpdir
    except Exception:
        pass


def _run(in_maps, trace=False, tmpdir=None):
    from concourse.bass_utils import run_bass_kernel_spmd

    if "nc" not in _CACHE:
        _setup_axon_profile_hook()
        _CACHE["nc"] = _build()
    kw = dict(trace=True, tmpdir=tmpdir) if trace else {}
    return run_bass_kernel_spmd(_CACHE["nc"], in_maps,
                                core_ids=list(range(8)), **kw)


def kernel(**inputs):
    in_maps = _make_in_maps(**inputs)
    res = _run(in_maps)
    return _unshard(res.results)


def kernel_traced(tmpdir, **inputs):
    in_maps = _make_in_maps(**inputs)
    res = _run(in_maps, trace=True, tmpdir=tmpdir)
    return _unshard(res.results), res



# revision 8
# speedup vs baseline: 1.0034x; 1.0034x over previous
"""Trainium2 Bass kernel for nn_BiLSTM_5970004542177.

Model: 2-layer bidirectional LSTM (Keras gate order i,f,g,o), B=128, T=256,
D=U=256, residual on layer 1, merge_mode='ave'.

Device mapping (8 NeuronCores, SPMD single program, no cross-core comm):
  core = (direction, batch quarter): cores 0-3 forward, 4-7 backward
  (backward = time-reversed input, host un-reverses the output).

Each core runs BOTH layers of its chain at B=32 in transposed layout
(partitions = units, free = batch), chunk-interleaved: layer-1 chunk j-1 is
emitted right after layer-0 chunk j, so the two recurrences' serial
dependency chains overlap across engines.  The input projection W^T x + b
is fused into the same PSUM accumulation group as the per-step recurrence
matmuls (bias rides a third K-tile against a constant ones-row).  Layer 1
reads layer 0's h history directly from SBUF and emits
out = 0.5*(h1 + h0); the host adds fw+bw shards and restores (B, T, U).
"""
import sys

if "/opt/trn_rl_repo" not in sys.path:
    sys.path.insert(0, "/opt/trn_rl_repo")

import numpy as np
import ml_dtypes

B = 32            # per-core batch (128 / 4 quarters)
T = 256
D = 256
U = 256
C = 32            # chunk length (steps)
NC = T // C
GS = 4            # steps per PSUM group
NKW = 3           # proj K-tiles (2 data + bias row)
NKR = 2
NM = 8
CB = C * B

_CACHE = {}


class _Unit:
    """Emission helper for one LSTM layer; supports fine interleaving."""

    def __init__(self, nc, mybir, pools, tag, W_sb, R_sb, rhs_fn, hist_ap,
                 h_prev0, c_sb):
        self.nc, self.mybir, self.pools = nc, mybir, pools
        self.tag = tag
        self.W_sb, self.R_sb = W_sb, R_sb
        self.rhs_fn, self.hist_ap = rhs_fn, hist_ap
        self.h_prev0, self.c_sb = h_prev0, c_sb
        self.zp = None

    def _proj_mms(self, zp, g, m_lo, m_hi):
        nc = self.nc
        for m in range(m_lo, m_hi):
            for k in range(NKW):
                nc.tensor.matmul(
                    zp[:, m, :],
                    self.W_sb[:, (m * NKW + k) * 128:(m * NKW + k + 1) * 128],
                    self.rhs_fn(k, g),
                    start=(k == 0 and (m * GS * B) % 512 == 0), stop=False,
                    skip_group_check=True,
                )

    def _new_zp(self):
        F32 = self.mybir.dt.float32
        zp_t = self.pools["psum"].tile([128, NM, GS * B], F32,
                                       tag="zp" + self.tag)
        return zp_t

    def emit_proj(self, g):
        self.zp = self._new_zp()
        self._proj_mms(self.zp, g, 0, NM)

    def emit_proj_slice(self, g, sl):
        """Emit a quarter of group g's projection (2 M-strips); used to fill
        PE stalls during the previous group's recurrence steps."""
        if sl == 0:
            self.zp_next = self._new_zp()
        self._proj_mms(self.zp_next, g, 2 * sl, 2 * sl + 2)

    def advance_group(self):
        self.zp = self.zp_next

    def emit_rec_mms(self, g, sl):
        nc = self.nc
        s = g * GS + sl
        h_prev = self.h_prev0 if s == 0 else self.hist_ap[:, s - 1]
        for m in range(NM):
            for k in range(NKR):
                nc.tensor.matmul(
                    self.zp[:, m, sl * B:(sl + 1) * B],
                    self.R_sb[:, (m * NKR + k) * 128:(m * NKR + k + 1) * 128],
                    h_prev[:, k, :],
                    start=False, stop=(k == NKR - 1),
                    skip_group_check=True,
                )

    def emit_sigmoid(self, g, sl):
        nc, mybir = self.nc, self.mybir
        BF16 = mybir.dt.bfloat16
        SIG = mybir.ActivationFunctionType.Sigmoid
        work = self.pools["work"]
        self.gt = work.tile([128, NM, B], BF16, tag="gt" + self.tag)
        zs = self.zp[:, :, sl * B:(sl + 1) * B]
        # all four gates through one sigmoid; the g columns were pre-scaled
        # by 2 on the host so tanh(zg) = 2*sigmoid(2 zg) - 1 = 2*gt_g - 1
        nc.scalar.activation(self.gt[:], zs[:], SIG)

    def emit_t1_cmul(self):
        nc, mybir = self.nc, self.mybir
        BF16 = mybir.dt.bfloat16
        MULT = mybir.AluOpType.mult
        work = self.pools["work"]
        gt = self.gt
        self.t1 = work.tile([128, 2, B], BF16, tag="t1" + self.tag)
        # c = f*c + i*(2*sg - 1) = f*c + (2*(i*sg) - i)
        # t1 = i*sg as STT to hit the 4x DVE perf mode (all-bf16, SBUF)
        nc.vector.scalar_tensor_tensor(self.t1[:], gt[:, 0:2, :], 1.0,
                                       gt[:, 4:6, :], op0=MULT, op1=MULT)
        # c *= f on the otherwise-idle GpSimd engine, overlaps t1/t2
        nc.gpsimd.tensor_tensor(self.c_sb[:], self.c_sb[:], gt[:, 2:4, :],
                                op=MULT)

    def emit_t2(self):
        nc, mybir = self.nc, self.mybir
        BF16 = mybir.dt.bfloat16
        MULT = mybir.AluOpType.mult
        SUB = mybir.AluOpType.subtract
        work = self.pools["work"]
        self.t2 = work.tile([128, 2, B], BF16, tag="t2" + self.tag)
        nc.vector.scalar_tensor_tensor(self.t2[:], self.t1[:], 2.0,
                                       self.gt[:, 0:2, :], op0=MULT, op1=SUB)

    def emit_cadd(self):
        nc, mybir = self.nc, self.mybir
        ADD = mybir.AluOpType.add
        nc.vector.tensor_tensor(self.c_sb[:], self.c_sb[:], self.t2[:],
                                op=ADD)

    def emit_tanh(self):
        nc, mybir = self.nc, self.mybir
        BF16 = mybir.dt.bfloat16
        TANH = mybir.ActivationFunctionType.Tanh
        work = self.pools["work"]
        self.tct = work.tile([128, 2, B], BF16, tag="tc" + self.tag)
        nc.scalar.activation(self.tct[:], self.c_sb[:], TANH)

    def emit_ho(self, g, sl):
        nc, mybir = self.nc, self.mybir
        MULT = mybir.AluOpType.mult
        s = g * GS + sl
        nc.vector.scalar_tensor_tensor(self.hist_ap[:, s], self.gt[:, 6:8, :],
                                       1.0, self.tct[:], op0=MULT, op1=MULT)


def _build():
    import concourse.bacc as bacc
    import concourse.tile as tile
    from concourse import mybir

    F32 = mybir.dt.float32
    BF16 = mybir.dt.bfloat16
    ADD = mybir.AluOpType.add

    nc = bacc.Bacc("TRN2", target_bir_lowering=False, debug=False)
    W0d = nc.dram_tensor("Wp0", [128, NKW * NM * 128], BF16,
                         kind="ExternalInput")
    R0d = nc.dram_tensor("Rp0", [128, NKR * NM * 128], BF16,
                         kind="ExternalInput")
    W1d = nc.dram_tensor("Wp1", [128, NKW * NM * 128], BF16,
                         kind="ExternalInput")
    R1d = nc.dram_tensor("Rp1", [128, NKR * NM * 128], BF16,
                         kind="ExternalInput")
    Xd = nc.dram_tensor("Xp", [128, 2, T * B], BF16, kind="ExternalInput")
    OutD = nc.dram_tensor("Out", [128, 2, T * 2 * B], BF16,
                          kind="ExternalOutput")

    with tile.TileContext(nc) as tc:
        with (
            tc.tile_pool(name="const", bufs=1) as const,
            tc.tile_pool(name="state", bufs=1) as state,
            tc.tile_pool(name="work", bufs=6) as work,
            tc.tile_pool(name="psum", bufs=2, space="PSUM") as psum,
        ):
            W0 = const.tile([128, NKW * NM * 128], BF16)
            R0 = const.tile([128, NKR * NM * 128], BF16)
            W1 = const.tile([128, NKW * NM * 128], BF16)
            R1 = const.tile([128, NKR * NM * 128], BF16)
            nc.sync.dma_start(out=W0[:], in_=W0d[:])
            nc.sync.dma_start(out=R0[:], in_=R0d[:])
            nc.sync.dma_start(out=W1[:], in_=W1d[:])
            nc.sync.dma_start(out=R1[:], in_=R1d[:])

            xin = const.tile([128, 2, T * B], BF16)
            # per-chunk slices so chunk 0's matmuls start after 1/NC of the
            # input transfer instead of the whole 4 MB
            for jj in range(NC):
                nc.sync.dma_start(out=xin[:, :, jj * CB:(jj + 1) * CB],
                                  in_=Xd[:, :, jj * CB:(jj + 1) * CB])
            ones = const.tile([128, GS * B], BF16)
            nc.vector.memset(ones[:], 0.0)
            nc.vector.memset(ones[0:1, :], 1.0)

            hist0 = state.tile([128, T, 2, B], BF16)
            hist1 = state.tile([128, T, 2, B], BF16)
            h00 = state.tile([128, 2, B], BF16)
            c0 = state.tile([128, 2, B], F32)
            c1 = state.tile([128, 2, B], F32)
            nc.vector.memset(h00[:], 0.0)
            nc.vector.memset(c0[:], 0.0)
            nc.vector.memset(c1[:], 0.0)

            pools = {"psum": psum, "work": work}

            def rhs_l0(j):
                def fn(k, g):
                    if k < 2:
                        a = j * C + g * GS
                        return xin[:, k, a * B:(a + GS) * B]
                    return ones[:]
                return fn

            def rhs_l1(j):
                def fn(k, g):
                    if k < 2:
                        a = j * C + g * GS
                        return hist0[:, a:a + GS, k, :]
                    return ones[:]
                return fn

            NG = C // GS
            # u1's (layer-1) tanh/ho for step s are deferred to slot s+1 so
            # the Act queue per slot reads [sig0(s), tanh1(s-1), sig1(s),
            # tanh0(s)]: each unit's step-to-step cycle then closes through
            # only its own tail instead of threading both units' tails into
            # one serial Act cycle.
            pending = None  # (unit, g, sl) whose tanh/ho still to emit
            for j in range(NC + 1):
                u0 = u1 = None
                if j < NC:
                    u0 = _Unit(nc, mybir, pools, "a", W0, R0, rhs_l0(j),
                               hist0[:, j * C:(j + 1) * C],
                               h00 if j == 0 else hist0[:, j * C - 1], c0)
                if j >= 1:
                    i = j - 1
                    u1 = _Unit(nc, mybir, pools, "b", W1, R1, rhs_l1(i),
                               hist1[:, i * C:(i + 1) * C],
                               h00 if i == 0 else hist1[:, i * C - 1], c1)
                for g in range(NG):
                    for u in (u0, u1):
                        if u is None:
                            continue
                        if g == 0:
                            u.emit_proj(0)
                        else:
                            u.advance_group()
                    for sl in range(GS):
                        if u0 is not None:
                            u0.emit_rec_mms(g, sl)
                            u0.emit_sigmoid(g, sl)
                            u0.emit_t1_cmul()
                            u0.emit_t2()
                            u0.emit_cadd()
                        if pending is not None:
                            pu, pg, psl = pending
                            pu.emit_tanh()
                            pu.emit_ho(pg, psl)
                            pending = None
                        if u1 is not None:
                            u1.emit_rec_mms(g, sl)
                            u1.emit_sigmoid(g, sl)
                            u1.emit_t1_cmul()
                            u1.emit_t2()
                            u1.emit_cadd()
                            pending = (u1, g, sl)
                        if u0 is not None:
                            u0.emit_tanh()
                            u0.emit_ho(g, sl)
                        if g + 1 < NG:
                            for u in (u0, u1):
                                if u is not None:
                                    u.emit_proj_slice(g + 1, sl)
                # ship both layers' raw h chunks; the host does h0+h1.
                # hist0 chunk j completes this iteration; hist1 chunk j-1
                # finishes only after the deferred tail flushes next
                # iteration, so its DMA lags one more chunk.
                if u0 is not None:
                    nc.sync.dma_start(
                        out=OutD[:, 0, j * C * 2 * B:(j + 1) * C * 2 * B],
                        in_=hist0[:, j * C:(j + 1) * C].rearrange(
                            "p c k b -> p (c k b)"))
                if j >= 2:
                    i = j - 2
                    nc.sync.dma_start(
                        out=OutD[:, 1, i * C * 2 * B:(i + 1) * C * 2 * B],
                        in_=hist1[:, i * C:(i + 1) * C].rearrange(
                            "p c k b -> p (c k b)"))
            # flush the last pending tail and the final hist1 chunk
            if pending is not None:
                pu, pg, psl = pending
                pu.emit_tanh()
                pu.emit_ho(pg, psl)
            nc.sync.dma_start(
                out=OutD[:, 1, (NC - 1) * C * 2 * B:NC * C * 2 * B],
                in_=hist1[:, (NC - 1) * C:NC * C].rearrange(
                    "p c k b -> p (c k b)"))

    nc.compile()
    return nc


# ------------------------------------------------------------- host packing
def _pack_W_aug(W, b):
    out = np.zeros((128, NKW * NM * 128), np.float32)
    for m in range(NM):
        for k in range(NKW):
            col = (m * NKW + k) * 128
            if k < 2:
                out[:, col:col + 128] = W[k * 128:(k + 1) * 128,
                                          m * 128:(m + 1) * 128]
            else:
                out[0, col:col + 128] = b[m * 128:(m + 1) * 128]
    return out.astype(ml_dtypes.bfloat16)


def _pack_R(R):
    out = np.zeros((128, NKR * NM * 128), np.float32)
    for m in range(NM):
        for k in range(NKR):
            col = (m * NKR + k) * 128
            out[:, col:col + 128] = R[k * 128:(k + 1) * 128,
                                      m * 128:(m + 1) * 128]
    return out.astype(ml_dtypes.bfloat16)


def _pack_x(xs):
    """xs (B, T, D) -> [128, 2, T*B] bf16 (k-tile, t-major cols)."""
    xt = np.ascontiguousarray(np.transpose(xs, (2, 1, 0))).reshape(D, T * B)
    out = np.empty((128, 2, T * B), np.float32)
    out[:, 0, :] = xt[0:128]
    out[:, 1, :] = xt[128:256]
    return out.astype(ml_dtypes.bfloat16)


def _make_in_maps(x, kernels_fw, rec_fw, bias_fw, kernels_bw, rec_bw, bias_bw):
    x = np.asarray(x, np.float32)
    xr = x[:, ::-1, :]
    def g2(a):
        a = np.array(a, np.float32)
        a[..., 2 * U:3 * U] *= 2.0
        return a

    packs = {}
    for d, Ws, Rs, bs in (("fw", kernels_fw, rec_fw, bias_fw),
                          ("bw", kernels_bw, rec_bw, bias_bw)):
        packs[d] = [
            (_pack_W_aug(g2(Ws[li]), g2(bs[li])), _pack_R(g2(Rs[li])))
            for li in range(2)
        ]
    in_maps = []
    for core in range(8):
        d = "fw" if core < 4 else "bw"
        q = core % 4
        xs = (x if d == "fw" else xr)[q * B:(q + 1) * B]
        (W0, R0), (W1, R1) = packs[d]
        in_maps.append({"Wp0": W0, "Rp0": R0, "Wp1": W1, "Rp1": R1,
                        "Xp": _pack_x(xs)})
    return in_maps


def _unshard(results):
    full = np.zeros((128, T, U), np.float32)
    for core in range(8):
        d_rev = core >= 4
        q = core % 4
        raw = np.asarray(results[core]["Out"], dtype=np.float32)
        o = (raw[:, 0, :] + raw[:, 1, :]).reshape(128, T, 2, B)
        o = np.transpose(o, (3, 1, 2, 0)).reshape(B, T, U)
        if d_rev:
            o = o[:, ::-1, :]
        full[q * B:(q + 1) * B] += o
    full *= 0.5
    return full


def _setup_axon_profile_hook():
    try:
        import types
        import antenv
        mod = sys.modules.get("antenv.axon_hooks")
        if mod is None:
            mod = types.ModuleType("antenv.axon_hooks")
            holder = {"hook": None}
            mod.set_axon_ntff_profile_hook = lambda h: holder.update(hook=h)
            mod.get_axon_ntff_profile_hook = lambda: holder["hook"]
            sys.modules["antenv.axon_hooks"] = mod
            antenv.axon_hooks = mod
        from trn_agent_boot.trn_boot import _ntff_profile_via_ctypes
        hook = _ntff_profile_via_ctypes("/opt/axon/libaxon_pjrt.so")
        if hook is not None:
            mod.set_axon_ntff_profile_hook(hook)
        import concourse.bass_utils as bass_utils
        bass_utils.upload_artifacts = lambda tmpdir: tmpdir
    except Exception:
        pass


def _run(in_maps, trace=False, tmpdir=None):
    from concourse.bass_utils import run_bass_kernel_spmd

    if "nc" not in _CACHE:
        _setup_axon_profile_hook()
        _CACHE["nc"] = _build()
    kw = dict(trace=True, tmpdir=tmpdir) if trace else {}
    return run_bass_kernel_spmd(_CACHE["nc"], in_maps,
                                core_ids=list(range(8)), **kw)


def kernel(**inputs):
    in_maps = _make_in_maps(**inputs)
    res = _run(in_maps)
    return _unshard(res.results)


def kernel_traced(tmpdir, **inputs):
    in_maps = _make_in_maps(**inputs)
    res = _run(in_maps, trace=True, tmpdir=tmpdir)
    return _unshard(res.results), res



# revision 9
# speedup vs baseline: 1.0099x; 1.0065x over previous
"""Trainium2 Bass kernel for nn_BiLSTM_5970004542177.

Model: 2-layer bidirectional LSTM (Keras gate order i,f,g,o), B=128, T=256,
D=U=256, residual on layer 1, merge_mode='ave'.

Device mapping (8 NeuronCores, SPMD single program, no cross-core comm):
  core = (direction, batch quarter): cores 0-3 forward, 4-7 backward
  (backward = time-reversed input, host un-reverses the output).

Each core runs BOTH layers of its chain at B=32 in transposed layout
(partitions = units, free = batch), chunk-interleaved: layer-1 chunk j-1 is
emitted right after layer-0 chunk j, so the two recurrences' serial
dependency chains overlap across engines.  The input projection W^T x + b
is fused into the same PSUM accumulation group as the per-step recurrence
matmuls (bias rides a third K-tile against a constant ones-row).  Layer 1
reads layer 0's h history directly from SBUF and emits
out = 0.5*(h1 + h0); the host adds fw+bw shards and restores (B, T, U).
"""
import sys

if "/opt/trn_rl_repo" not in sys.path:
    sys.path.insert(0, "/opt/trn_rl_repo")

import numpy as np
import ml_dtypes

B = 32            # per-core batch (128 / 4 quarters)
T = 256
D = 256
U = 256
C = 32            # chunk length (steps)
NC = T // C
GS = 4            # steps per PSUM group
NKW = 3           # proj K-tiles (2 data + bias row)
NKR = 2
NM = 8
CB = C * B

_CACHE = {}


class _Unit:
    """Emission helper for one LSTM layer; supports fine interleaving."""

    def __init__(self, nc, mybir, pools, tag, W_sb, R_sb, rhs_fn, hist_ap,
                 h_prev0, c_sb):
        self.nc, self.mybir, self.pools = nc, mybir, pools
        self.tag = tag
        self.W_sb, self.R_sb = W_sb, R_sb
        self.rhs_fn, self.hist_ap = rhs_fn, hist_ap
        self.h_prev0, self.c_sb = h_prev0, c_sb
        self.zp = None

    def _proj_mms(self, zp, g, m_lo, m_hi):
        nc = self.nc
        for m in range(m_lo, m_hi):
            for k in range(NKW):
                nc.tensor.matmul(
                    zp[:, m, :],
                    self.W_sb[:, (m * NKW + k) * 128:(m * NKW + k + 1) * 128],
                    self.rhs_fn(k, g),
                    start=(k == 0 and (m * GS * B) % 512 == 0), stop=False,
                    skip_group_check=True,
                )

    def _new_zp(self):
        F32 = self.mybir.dt.float32
        zp_t = self.pools["psum"].tile([128, NM, GS * B], F32,
                                       tag="zp" + self.tag)
        return zp_t

    def emit_proj(self, g):
        self.zp = self._new_zp()
        self._proj_mms(self.zp, g, 0, NM)

    def emit_proj_slice(self, g, sl):
        """Emit a quarter of group g's projection (2 M-strips); used to fill
        PE stalls during the previous group's recurrence steps."""
        if sl == 0:
            self.zp_next = self._new_zp()
        self._proj_mms(self.zp_next, g, 2 * sl, 2 * sl + 2)

    def advance_group(self):
        self.zp = self.zp_next

    def emit_rec_mms(self, g, sl):
        nc = self.nc
        s = g * GS + sl
        h_prev = self.h_prev0 if s == 0 else self.hist_ap[:, s - 1]
        for m in range(NM):
            for k in range(NKR):
                nc.tensor.matmul(
                    self.zp[:, m, sl * B:(sl + 1) * B],
                    self.R_sb[:, (m * NKR + k) * 128:(m * NKR + k + 1) * 128],
                    h_prev[:, k, :],
                    start=False, stop=(k == NKR - 1),
                    skip_group_check=True,
                )

    def emit_sigmoid(self, g, sl):
        nc, mybir = self.nc, self.mybir
        BF16 = mybir.dt.bfloat16
        SIG = mybir.ActivationFunctionType.Sigmoid
        work = self.pools["work"]
        self.gt = work.tile([128, NM, B], BF16, tag="gt" + self.tag)
        zs = self.zp[:, :, sl * B:(sl + 1) * B]
        # all four gates through one sigmoid; the g columns were pre-scaled
        # by 2 on the host so tanh(zg) = 2*sigmoid(2 zg) - 1 = 2*gt_g - 1
        nc.scalar.activation(self.gt[:], zs[:], SIG)

    def emit_t1_cmul(self):
        nc, mybir = self.nc, self.mybir
        BF16 = mybir.dt.bfloat16
        MULT = mybir.AluOpType.mult
        work = self.pools["work"]
        gt = self.gt
        self.t1 = work.tile([128, 2, B], BF16, tag="t1" + self.tag)
        # c = f*c + i*(2*sg - 1) = f*c + (2*(i*sg) - i)
        # t1 = i*sg as STT to hit the 4x DVE perf mode (all-bf16, SBUF)
        nc.vector.scalar_tensor_tensor(self.t1[:], gt[:, 0:2, :], 1.0,
                                       gt[:, 4:6, :], op0=MULT, op1=MULT)
        # c *= f on the otherwise-idle GpSimd engine, overlaps t1/t2
        nc.gpsimd.tensor_tensor(self.c_sb[:], self.c_sb[:], gt[:, 2:4, :],
                                op=MULT)

    def emit_t2(self):
        nc, mybir = self.nc, self.mybir
        BF16 = mybir.dt.bfloat16
        MULT = mybir.AluOpType.mult
        SUB = mybir.AluOpType.subtract
        work = self.pools["work"]
        self.t2 = work.tile([128, 2, B], BF16, tag="t2" + self.tag)
        nc.vector.scalar_tensor_tensor(self.t2[:], self.t1[:], 2.0,
                                       self.gt[:, 0:2, :], op0=MULT, op1=SUB)

    def emit_cadd(self):
        nc, mybir = self.nc, self.mybir
        ADD = mybir.AluOpType.add
        nc.vector.tensor_tensor(self.c_sb[:], self.c_sb[:], self.t2[:],
                                op=ADD)

    def emit_tanh(self):
        nc, mybir = self.nc, self.mybir
        BF16 = mybir.dt.bfloat16
        TANH = mybir.ActivationFunctionType.Tanh
        work = self.pools["work"]
        self.tct = work.tile([128, 2, B], BF16, tag="tc" + self.tag)
        nc.scalar.activation(self.tct[:], self.c_sb[:], TANH)

    def emit_ho(self, g, sl):
        nc, mybir = self.nc, self.mybir
        MULT = mybir.AluOpType.mult
        s = g * GS + sl
        nc.vector.scalar_tensor_tensor(self.hist_ap[:, s], self.gt[:, 6:8, :],
                                       1.0, self.tct[:], op0=MULT, op1=MULT)


def _build():
    import concourse.bacc as bacc
    import concourse.tile as tile
    from concourse import mybir

    F32 = mybir.dt.float32
    BF16 = mybir.dt.bfloat16
    ADD = mybir.AluOpType.add

    nc = bacc.Bacc("TRN2", target_bir_lowering=False, debug=False)
    W0d = nc.dram_tensor("Wp0", [128, NKW * NM * 128], BF16,
                         kind="ExternalInput")
    R0d = nc.dram_tensor("Rp0", [128, NKR * NM * 128], BF16,
                         kind="ExternalInput")
    W1d = nc.dram_tensor("Wp1", [128, NKW * NM * 128], BF16,
                         kind="ExternalInput")
    R1d = nc.dram_tensor("Rp1", [128, NKR * NM * 128], BF16,
                         kind="ExternalInput")
    Xd = nc.dram_tensor("Xp", [128, 2, T * B], BF16, kind="ExternalInput")
    OutD = nc.dram_tensor("Out", [128, 2, T * 2 * B], BF16,
                          kind="ExternalOutput")

    with tile.TileContext(nc) as tc:
        with (
            tc.tile_pool(name="const", bufs=1) as const,
            tc.tile_pool(name="state", bufs=1) as state,
            tc.tile_pool(name="work", bufs=6) as work,
            tc.tile_pool(name="psum", bufs=2, space="PSUM") as psum,
        ):
            W0 = const.tile([128, NKW * NM * 128], BF16)
            R0 = const.tile([128, NKR * NM * 128], BF16)
            W1 = const.tile([128, NKW * NM * 128], BF16)
            R1 = const.tile([128, NKR * NM * 128], BF16)
            nc.sync.dma_start(out=W0[:], in_=W0d[:])
            nc.sync.dma_start(out=R0[:], in_=R0d[:])
            nc.sync.dma_start(out=W1[:], in_=W1d[:])
            nc.sync.dma_start(out=R1[:], in_=R1d[:])

            xin = const.tile([128, 2, T * B], BF16)
            # per-chunk slices so chunk 0's matmuls start after 1/NC of the
            # input transfer instead of the whole 4 MB
            for jj in range(NC):
                nc.sync.dma_start(out=xin[:, :, jj * CB:(jj + 1) * CB],
                                  in_=Xd[:, :, jj * CB:(jj + 1) * CB])
            ones = const.tile([128, GS * B], BF16)
            nc.vector.memset(ones[:], 0.0)
            nc.vector.memset(ones[0:1, :], 1.0)

            hist0 = state.tile([128, T, 2, B], BF16)
            hist1 = state.tile([128, T, 2, B], BF16)
            h00 = state.tile([128, 2, B], BF16)
            c0 = state.tile([128, 2, B], BF16)
            c1 = state.tile([128, 2, B], BF16)
            nc.vector.memset(h00[:], 0.0)
            nc.vector.memset(c0[:], 0.0)
            nc.vector.memset(c1[:], 0.0)

            pools = {"psum": psum, "work": work}

            def rhs_l0(j):
                def fn(k, g):
                    if k < 2:
                        a = j * C + g * GS
                        return xin[:, k, a * B:(a + GS) * B]
                    return ones[:]
                return fn

            def rhs_l1(j):
                def fn(k, g):
                    if k < 2:
                        a = j * C + g * GS
                        return hist0[:, a:a + GS, k, :]
                    return ones[:]
                return fn

            NG = C // GS
            # u1's (layer-1) tanh/ho for step s are deferred to slot s+1 so
            # the Act queue per slot reads [sig0(s), tanh1(s-1), sig1(s),
            # tanh0(s)]: each unit's step-to-step cycle then closes through
            # only its own tail instead of threading both units' tails into
            # one serial Act cycle.
            pending = None  # (unit, g, sl) whose tanh/ho still to emit
            for j in range(NC + 1):
                u0 = u1 = None
                if j < NC:
                    u0 = _Unit(nc, mybir, pools, "a", W0, R0, rhs_l0(j),
                               hist0[:, j * C:(j + 1) * C],
                               h00 if j == 0 else hist0[:, j * C - 1], c0)
                if j >= 1:
                    i = j - 1
                    u1 = _Unit(nc, mybir, pools, "b", W1, R1, rhs_l1(i),
                               hist1[:, i * C:(i + 1) * C],
                               h00 if i == 0 else hist1[:, i * C - 1], c1)
                for g in range(NG):
                    for u in (u0, u1):
                        if u is None:
                            continue
                        if g == 0:
                            u.emit_proj(0)
                        else:
                            u.advance_group()
                    for sl in range(GS):
                        if u0 is not None:
                            u0.emit_rec_mms(g, sl)
                            u0.emit_sigmoid(g, sl)
                            u0.emit_t1_cmul()
                            u0.emit_t2()
                            u0.emit_cadd()
                        if pending is not None:
                            pu, pg, psl = pending
                            pu.emit_tanh()
                            pu.emit_ho(pg, psl)
                            pending = None
                        if u1 is not None:
                            u1.emit_rec_mms(g, sl)
                            u1.emit_sigmoid(g, sl)
                            u1.emit_t1_cmul()
                            u1.emit_t2()
                            u1.emit_cadd()
                            pending = (u1, g, sl)
                        if u0 is not None:
                            u0.emit_tanh()
                            u0.emit_ho(g, sl)
                        if g + 1 < NG:
                            for u in (u0, u1):
                                if u is not None:
                                    u.emit_proj_slice(g + 1, sl)
                # ship both layers' raw h chunks; the host does h0+h1.
                # hist0 chunk j completes this iteration; hist1 chunk j-1
                # finishes only after the deferred tail flushes next
                # iteration, so its DMA lags one more chunk.
                if u0 is not None:
                    nc.sync.dma_start(
                        out=OutD[:, 0, j * C * 2 * B:(j + 1) * C * 2 * B],
                        in_=hist0[:, j * C:(j + 1) * C].rearrange(
                            "p c k b -> p (c k b)"))
                if j >= 2:
                    i = j - 2
                    nc.sync.dma_start(
                        out=OutD[:, 1, i * C * 2 * B:(i + 1) * C * 2 * B],
                        in_=hist1[:, i * C:(i + 1) * C].rearrange(
                            "p c k b -> p (c k b)"))
            # flush the last pending tail and the final hist1 chunk
            if pending is not None:
                pu, pg, psl = pending
                pu.emit_tanh()
                pu.emit_ho(pg, psl)
            nc.sync.dma_start(
                out=OutD[:, 1, (NC - 1) * C * 2 * B:NC * C * 2 * B],
                in_=hist1[:, (NC - 1) * C:NC * C].rearrange(
                    "p c k b -> p (c k b)"))

    nc.compile()
    return nc


# ------------------------------------------------------------- host packing
def _pack_W_aug(W, b):
    out = np.zeros((128, NKW * NM * 128), np.float32)
    for m in range(NM):
        for k in range(NKW):
            col = (m * NKW + k) * 128
            if k < 2:
                out[:, col:col + 128] = W[k * 128:(k + 1) * 128,
                                          m * 128:(m + 1) * 128]
            else:
                out[0, col:col + 128] = b[m * 128:(m + 1) * 128]
    return out.astype(ml_dtypes.bfloat16)


def _pack_R(R):
    out = np.zeros((128, NKR * NM * 128), np.float32)
    for m in range(NM):
        for k in range(NKR):
            col = (m * NKR + k) * 128
            out[:, col:col + 128] = R[k * 128:(k + 1) * 128,
                                      m * 128:(m + 1) * 128]
    return out.astype(ml_dtypes.bfloat16)


def _pack_x(xs):
    """xs (B, T, D) -> [128, 2, T*B] bf16 (k-tile, t-major cols)."""
    xt = np.ascontiguousarray(np.transpose(xs, (2, 1, 0))).reshape(D, T * B)
    out = np.empty((128, 2, T * B), np.float32)
    out[:, 0, :] = xt[0:128]
    out[:, 1, :] = xt[128:256]
    return out.astype(ml_dtypes.bfloat16)


def _make_in_maps(x, kernels_fw, rec_fw, bias_fw, kernels_bw, rec_bw, bias_bw):
    x = np.asarray(x, np.float32)
    xr = x[:, ::-1, :]
    def g2(a):
        a = np.array(a, np.float32)
        a[..., 2 * U:3 * U] *= 2.0
        return a

    packs = {}
    for d, Ws, Rs, bs in (("fw", kernels_fw, rec_fw, bias_fw),
                          ("bw", kernels_bw, rec_bw, bias_bw)):
        packs[d] = [
            (_pack_W_aug(g2(Ws[li]), g2(bs[li])), _pack_R(g2(Rs[li])))
            for li in range(2)
        ]
    in_maps = []
    for core in range(8):
        d = "fw" if core < 4 else "bw"
        q = core % 4
        xs = (x if d == "fw" else xr)[q * B:(q + 1) * B]
        (W0, R0), (W1, R1) = packs[d]
        in_maps.append({"Wp0": W0, "Rp0": R0, "Wp1": W1, "Rp1": R1,
                        "Xp": _pack_x(xs)})
    return in_maps


def _unshard(results):
    full = np.zeros((128, T, U), np.float32)
    for core in range(8):
        d_rev = core >= 4
        q = core % 4
        raw = np.asarray(results[core]["Out"], dtype=np.float32)
        o = (raw[:, 0, :] + raw[:, 1, :]).reshape(128, T, 2, B)
        o = np.transpose(o, (3, 1, 2, 0)).reshape(B, T, U)
        if d_rev:
            o = o[:, ::-1, :]
        full[q * B:(q + 1) * B] += o
    full *= 0.5
    return full


def _setup_axon_profile_hook():
    try:
        import types
        import antenv
        mod = sys.modules.get("antenv.axon_hooks")
        if mod is None:
            mod = types.ModuleType("antenv.axon_hooks")
            holder = {"hook": None}
            mod.set_axon_ntff_profile_hook = lambda h: holder.update(hook=h)
            mod.get_axon_ntff_profile_hook = lambda: holder["hook"]
            sys.modules["antenv.axon_hooks"] = mod
            antenv.axon_hooks = mod
        from trn_agent_boot.trn_boot import _ntff_profile_via_ctypes
        hook = _ntff_profile_via_ctypes("/opt/axon/libaxon_pjrt.so")
        if hook is not None:
            mod.set_axon_ntff_profile_hook(hook)
        import concourse.bass_utils as bass_utils
        bass_utils.upload_artifacts = lambda tmpdir: tmpdir
    except Exception:
        pass


def _run(in_maps, trace=False, tmpdir=None):
    from concourse.bass_utils import run_bass_kernel_spmd

    if "nc" not in _CACHE:
        _setup_axon_profile_hook()
        _CACHE["nc"] = _build()
    kw = dict(trace=True, tmpdir=tmpdir) if trace else {}
    return run_bass_kernel_spmd(_CACHE["nc"], in_maps,
                                core_ids=list(range(8)), **kw)


def kernel(**inputs):
    in_maps = _make_in_maps(**inputs)
    res = _run(in_maps)
    return _unshard(res.results)


def kernel_traced(tmpdir, **inputs):
    in_maps = _make_in_maps(**inputs)
    res = _run(in_maps, trace=True, tmpdir=tmpdir)
    return _unshard(res.results), res



# revision 12
# speedup vs baseline: 1.0207x; 1.0107x over previous
"""Trainium2 Bass kernel for nn_BiLSTM_5970004542177.

Model: 2-layer bidirectional LSTM (Keras gate order i,f,g,o), B=128, T=256,
D=U=256, residual on layer 1, merge_mode='ave'.

Device mapping (8 NeuronCores, SPMD single program, no cross-core comm):
  core = (direction, batch quarter): cores 0-3 forward, 4-7 backward
  (backward = time-reversed input, host un-reverses the output).

Each core runs BOTH layers of its chain at B=32 in transposed layout
(partitions = units, free = batch), chunk-interleaved: layer-1 chunk j-1 is
emitted right after layer-0 chunk j, so the two recurrences' serial
dependency chains overlap across engines.  The input projection W^T x + b
is fused into the same PSUM accumulation group as the per-step recurrence
matmuls (bias rides a third K-tile against a constant ones-row).  Layer 1
reads layer 0's h history directly from SBUF and emits
out = 0.5*(h1 + h0); the host adds fw+bw shards and restores (B, T, U).
"""
import sys

if "/opt/trn_rl_repo" not in sys.path:
    sys.path.insert(0, "/opt/trn_rl_repo")

import numpy as np
import ml_dtypes

B = 32            # per-core batch (128 / 4 quarters)
T = 256
D = 256
U = 256
C = 32            # chunk length (steps)
NC = T // C
GS = 4            # steps per PSUM group
NKW = 3           # proj K-tiles (2 data + bias row)
NKR = 2
NM = 8
CB = C * B

_CACHE = {}


class _Unit:
    """Emission helper for one LSTM layer; supports fine interleaving."""

    def __init__(self, nc, mybir, pools, tag, W_sb, R_sb, rhs_fn, hist_ap,
                 h_prev0, c_sb):
        self.nc, self.mybir, self.pools = nc, mybir, pools
        self.tag = tag
        self.W_sb, self.R_sb = W_sb, R_sb
        self.rhs_fn, self.hist_ap = rhs_fn, hist_ap
        self.h_prev0, self.c_sb = h_prev0, c_sb
        self.zp = None

    def _proj_mms(self, zp, g, m_lo, m_hi):
        nc = self.nc
        for m in range(m_lo, m_hi):
            for k in range(NKW):
                nc.tensor.matmul(
                    zp[:, m, :],
                    self.W_sb[:, (m * NKW + k) * 128:(m * NKW + k + 1) * 128],
                    self.rhs_fn(k, g),
                    start=(k == 0 and (m * GS * B) % 512 == 0), stop=False,
                    skip_group_check=True,
                )

    def _new_zp(self):
        F32 = self.mybir.dt.float32
        zp_t = self.pools["psum"].tile([128, NM, GS * B], F32,
                                       tag="zp" + self.tag)
        return zp_t

    def emit_proj(self, g):
        self.zp = self._new_zp()
        self._proj_mms(self.zp, g, 0, NM)

    def emit_proj_slice(self, g, sl):
        """Emit a quarter of group g's projection (2 M-strips); used to fill
        PE stalls during the previous group's recurrence steps."""
        if sl == 0:
            self.zp_next = self._new_zp()
        self._proj_mms(self.zp_next, g, 2 * sl, 2 * sl + 2)

    def advance_group(self):
        self.zp = self.zp_next

    def emit_rec_mms(self, g, sl):
        nc = self.nc
        s = g * GS + sl
        h_prev = self.h_prev0 if s == 0 else self.hist_ap[:, s - 1]
        for m in range(NM):
            for k in range(NKR):
                nc.tensor.matmul(
                    self.zp[:, m, sl * B:(sl + 1) * B],
                    self.R_sb[:, (m * NKR + k) * 128:(m * NKR + k + 1) * 128],
                    h_prev[:, k, :],
                    start=False, stop=(k == NKR - 1),
                    skip_group_check=True,
                )

    def emit_sigmoid_a(self, g, sl):
        """Gates are host-packed in (i, g, f, o) strip order; half A covers
        i and g (strips 0:4) so the t1/t2 chain can start while the m4-7
        recurrence matmuls and sigmoid B are still in flight."""
        nc, mybir = self.nc, self.mybir
        BF16 = mybir.dt.bfloat16
        SIG = mybir.ActivationFunctionType.Sigmoid
        work = self.pools["work"]
        self.gt = work.tile([128, NM, B], BF16, tag="gt" + self.tag)
        zs = self.zp[:, :, sl * B:(sl + 1) * B]
        nc.scalar.activation(self.gt[:, 0:4, :], zs[:, 0:4, :], SIG)

    def emit_sigmoid_b(self, g, sl):
        nc, mybir = self.nc, self.mybir
        SIG = mybir.ActivationFunctionType.Sigmoid
        zs = self.zp[:, :, sl * B:(sl + 1) * B]
        nc.scalar.activation(self.gt[:, 4:8, :], zs[:, 4:8, :], SIG)

    def emit_t1(self):
        nc, mybir = self.nc, self.mybir
        BF16 = mybir.dt.bfloat16
        MULT = mybir.AluOpType.mult
        work = self.pools["work"]
        gt = self.gt
        self.t1 = work.tile([128, 2, B], BF16, tag="t1" + self.tag)
        # c = f*c + i*(2*sg - 1) = f*c + (2*(i*sg) - i); g pre-scaled by 2
        # on the host so tanh(zg) = 2*sigmoid(2 zg) - 1
        nc.vector.scalar_tensor_tensor(self.t1[:], gt[:, 0:2, :], 1.0,
                                       gt[:, 2:4, :], op0=MULT, op1=MULT)

    def emit_t2(self):
        nc, mybir = self.nc, self.mybir
        BF16 = mybir.dt.bfloat16
        MULT = mybir.AluOpType.mult
        SUB = mybir.AluOpType.subtract
        work = self.pools["work"]
        self.t2 = work.tile([128, 2, B], BF16, tag="t2" + self.tag)
        nc.vector.scalar_tensor_tensor(self.t2[:], self.t1[:], 2.0,
                                       self.gt[:, 0:2, :], op0=MULT, op1=SUB)

    def emit_cmul(self):
        nc, mybir = self.nc, self.mybir
        MULT = mybir.AluOpType.mult
        nc.vector.tensor_tensor(self.c_sb[:], self.c_sb[:],
                                self.gt[:, 4:6, :], op=MULT)

    def emit_cadd(self):
        nc, mybir = self.nc, self.mybir
        ADD = mybir.AluOpType.add
        nc.vector.tensor_tensor(self.c_sb[:], self.c_sb[:], self.t2[:],
                                op=ADD)

    def emit_tanh(self):
        nc, mybir = self.nc, self.mybir
        BF16 = mybir.dt.bfloat16
        TANH = mybir.ActivationFunctionType.Tanh
        work = self.pools["work"]
        self.tct = work.tile([128, 2, B], BF16, tag="tc" + self.tag)
        nc.scalar.activation(self.tct[:], self.c_sb[:], TANH)

    def emit_ho(self, g, sl):
        nc, mybir = self.nc, self.mybir
        MULT = mybir.AluOpType.mult
        s = g * GS + sl
        nc.vector.scalar_tensor_tensor(self.hist_ap[:, s], self.gt[:, 6:8, :],
                                       1.0, self.tct[:], op0=MULT, op1=MULT)


def _build():
    import concourse.bacc as bacc
    import concourse.tile as tile
    from concourse import mybir

    F32 = mybir.dt.float32
    BF16 = mybir.dt.bfloat16
    ADD = mybir.AluOpType.add

    nc = bacc.Bacc("TRN2", target_bir_lowering=False, debug=False)
    W0d = nc.dram_tensor("Wp0", [128, NKW * NM * 128], BF16,
                         kind="ExternalInput")
    R0d = nc.dram_tensor("Rp0", [128, NKR * NM * 128], BF16,
                         kind="ExternalInput")
    W1d = nc.dram_tensor("Wp1", [128, NKW * NM * 128], BF16,
                         kind="ExternalInput")
    R1d = nc.dram_tensor("Rp1", [128, NKR * NM * 128], BF16,
                         kind="ExternalInput")
    Xd = nc.dram_tensor("Xp", [128, 2, T * B], BF16, kind="ExternalInput")
    OutD = nc.dram_tensor("Out", [128, 2, T * 2 * B], BF16,
                          kind="ExternalOutput")

    with tile.TileContext(nc) as tc:
        with (
            tc.tile_pool(name="const", bufs=1) as const,
            tc.tile_pool(name="state", bufs=1) as state,
            tc.tile_pool(name="work", bufs=6) as work,
            tc.tile_pool(name="psum", bufs=2, space="PSUM") as psum,
        ):
            W0 = const.tile([128, NKW * NM * 128], BF16)
            R0 = const.tile([128, NKR * NM * 128], BF16)
            W1 = const.tile([128, NKW * NM * 128], BF16)
            R1 = const.tile([128, NKR * NM * 128], BF16)
            nc.sync.dma_start(out=W0[:], in_=W0d[:])
            nc.sync.dma_start(out=R0[:], in_=R0d[:])
            nc.sync.dma_start(out=W1[:], in_=W1d[:])
            nc.sync.dma_start(out=R1[:], in_=R1d[:])

            xin = const.tile([128, 2, T * B], BF16)
            # per-chunk slices so chunk 0's matmuls start after 1/NC of the
            # input transfer instead of the whole 4 MB
            for jj in range(NC):
                nc.sync.dma_start(out=xin[:, :, jj * CB:(jj + 1) * CB],
                                  in_=Xd[:, :, jj * CB:(jj + 1) * CB])
            ones = const.tile([128, GS * B], BF16)
            nc.vector.memset(ones[:], 0.0)
            nc.vector.memset(ones[0:1, :], 1.0)

            hist0 = state.tile([128, T, 2, B], BF16)
            hist1 = state.tile([128, T, 2, B], BF16)
            h00 = state.tile([128, 2, B], BF16)
            c0 = state.tile([128, 2, B], BF16)
            c1 = state.tile([128, 2, B], BF16)
            nc.vector.memset(h00[:], 0.0)
            nc.vector.memset(c0[:], 0.0)
            nc.vector.memset(c1[:], 0.0)

            pools = {"psum": psum, "work": work}

            def rhs_l0(j):
                def fn(k, g):
                    if k < 2:
                        a = j * C + g * GS
                        return xin[:, k, a * B:(a + GS) * B]
                    return ones[:]
                return fn

            def rhs_l1(j):
                def fn(k, g):
                    if k < 2:
                        a = j * C + g * GS
                        return hist0[:, a:a + GS, k, :]
                    return ones[:]
                return fn

            NG = C // GS
            # u1's (layer-1) tanh/ho for step s are deferred to slot s+1 so
            # the Act queue per slot reads [sig0(s), tanh1(s-1), sig1(s),
            # tanh0(s)]: each unit's step-to-step cycle then closes through
            # only its own tail instead of threading both units' tails into
            # one serial Act cycle.
            pending = None  # (unit, g, sl) whose tanh/ho still to emit
            for j in range(NC + 1):
                u0 = u1 = None
                if j < NC:
                    u0 = _Unit(nc, mybir, pools, "a", W0, R0, rhs_l0(j),
                               hist0[:, j * C:(j + 1) * C],
                               h00 if j == 0 else hist0[:, j * C - 1], c0)
                if j >= 1:
                    i = j - 1
                    u1 = _Unit(nc, mybir, pools, "b", W1, R1, rhs_l1(i),
                               hist1[:, i * C:(i + 1) * C],
                               h00 if i == 0 else hist1[:, i * C - 1], c1)
                for g in range(NG):
                    for u in (u0, u1):
                        if u is None:
                            continue
                        if g == 0:
                            u.emit_proj(0)
                        else:
                            u.advance_group()
                    for sl in range(GS):
                        if u0 is not None:
                            u0.emit_rec_mms(g, sl)
                            u0.emit_sigmoid_a(g, sl)
                            u0.emit_sigmoid_b(g, sl)
                            u0.emit_t1()
                            u0.emit_t2()
                            u0.emit_cmul()
                            u0.emit_cadd()
                        if pending is not None:
                            pu, pg, psl = pending
                            pu.emit_tanh()
                            pu.emit_ho(pg, psl)
                            pending = None
                        if u1 is not None:
                            u1.emit_rec_mms(g, sl)
                            u1.emit_sigmoid_a(g, sl)
                            u1.emit_sigmoid_b(g, sl)
                            u1.emit_t1()
                            u1.emit_t2()
                            u1.emit_cmul()
                            u1.emit_cadd()
                            pending = (u1, g, sl)
                        if u0 is not None:
                            u0.emit_tanh()
                            u0.emit_ho(g, sl)
                        if g + 1 < NG:
                            for u in (u0, u1):
                                if u is not None:
                                    u.emit_proj_slice(g + 1, sl)
                # ship both layers' raw h chunks; the host does h0+h1.
                # hist0 chunk j completes this iteration; hist1 chunk j-1
                # finishes only after the deferred tail flushes next
                # iteration, so its DMA lags one more chunk.
                if u0 is not None:
                    nc.sync.dma_start(
                        out=OutD[:, 0, j * C * 2 * B:(j + 1) * C * 2 * B],
                        in_=hist0[:, j * C:(j + 1) * C].rearrange(
                            "p c k b -> p (c k b)"))
                if j >= 2:
                    i = j - 2
                    nc.sync.dma_start(
                        out=OutD[:, 1, i * C * 2 * B:(i + 1) * C * 2 * B],
                        in_=hist1[:, i * C:(i + 1) * C].rearrange(
                            "p c k b -> p (c k b)"))
            # flush the last pending tail and the final hist1 chunk
            if pending is not None:
                pu, pg, psl = pending
                pu.emit_tanh()
                pu.emit_ho(pg, psl)
            nc.sync.dma_start(
                out=OutD[:, 1, (NC - 1) * C * 2 * B:NC * C * 2 * B],
                in_=hist1[:, (NC - 1) * C:NC * C].rearrange(
                    "p c k b -> p (c k b)"))

    nc.compile()
    return nc


# ------------------------------------------------------------- host packing
def _pack_W_aug(W, b):
    out = np.zeros((128, NKW * NM * 128), np.float32)
    for m in range(NM):
        for k in range(NKW):
            col = (m * NKW + k) * 128
            if k < 2:
                out[:, col:col + 128] = W[k * 128:(k + 1) * 128,
                                          m * 128:(m + 1) * 128]
            else:
                out[0, col:col + 128] = b[m * 128:(m + 1) * 128]
    return out.astype(ml_dtypes.bfloat16)


def _pack_R(R):
    out = np.zeros((128, NKR * NM * 128), np.float32)
    for m in range(NM):
        for k in range(NKR):
            col = (m * NKR + k) * 128
            out[:, col:col + 128] = R[k * 128:(k + 1) * 128,
                                      m * 128:(m + 1) * 128]
    return out.astype(ml_dtypes.bfloat16)


def _pack_x(xs):
    """xs (B, T, D) -> [128, 2, T*B] bf16 (k-tile, t-major cols)."""
    xt = np.ascontiguousarray(np.transpose(xs, (2, 1, 0))).reshape(D, T * B)
    out = np.empty((128, 2, T * B), np.float32)
    out[:, 0, :] = xt[0:128]
    out[:, 1, :] = xt[128:256]
    return out.astype(ml_dtypes.bfloat16)


def _make_in_maps(x, kernels_fw, rec_fw, bias_fw, kernels_bw, rec_bw, bias_bw):
    x = np.asarray(x, np.float32)
    xr = x[:, ::-1, :]
    def g2(a):
        """Pre-scale the g gate by 2 (tanh-via-sigmoid trick), then permute
        gate blocks (i,f,g,o) -> (i,g,f,o) so each sigmoid half covers a
        contiguous strip range on device."""
        a = np.array(a, np.float32)
        a[..., 2 * U:3 * U] *= 2.0
        return np.concatenate([a[..., 0:U], a[..., 2 * U:3 * U],
                               a[..., U:2 * U], a[..., 3 * U:4 * U]], axis=-1)

    packs = {}
    for d, Ws, Rs, bs in (("fw", kernels_fw, rec_fw, bias_fw),
                          ("bw", kernels_bw, rec_bw, bias_bw)):
        packs[d] = [
            (_pack_W_aug(g2(Ws[li]), g2(bs[li])), _pack_R(g2(Rs[li])))
            for li in range(2)
        ]
    in_maps = []
    for core in range(8):
        d = "fw" if core < 4 else "bw"
        q = core % 4
        xs = (x if d == "fw" else xr)[q * B:(q + 1) * B]
        (W0, R0), (W1, R1) = packs[d]
        in_maps.append({"Wp0": W0, "Rp0": R0, "Wp1": W1, "Rp1": R1,
                        "Xp": _pack_x(xs)})
    return in_maps


def _unshard(results):
    full = np.zeros((128, T, U), np.float32)
    for core in range(8):
        d_rev = core >= 4
        q = core % 4
        raw = np.asarray(results[core]["Out"], dtype=np.float32)
        o = (raw[:, 0, :] + raw[:, 1, :]).reshape(128, T, 2, B)
        o = np.transpose(o, (3, 1, 2, 0)).reshape(B, T, U)
        if d_rev:
            o = o[:, ::-1, :]
        full[q * B:(q + 1) * B] += o
    full *= 0.5
    return full


def _setup_axon_profile_hook():
    try:
        import types
        import antenv
        mod = sys.modules.get("antenv.axon_hooks")
        if mod is None:
            mod = types.ModuleType("antenv.axon_hooks")
            holder = {"hook": None}
            mod.set_axon_ntff_profile_hook = lambda h: holder.update(hook=h)
            mod.get_axon_ntff_profile_hook = lambda: holder["hook"]
            sys.modules["antenv.axon_hooks"] = mod
            antenv.axon_hooks = mod
        from trn_agent_boot.trn_boot import _ntff_profile_via_ctypes
        hook = _ntff_profile_via_ctypes("/opt/axon/libaxon_pjrt.so")
        if hook is not None:
            mod.set_axon_ntff_profile_hook(hook)
        import concourse.bass_utils as bass_utils
        bass_utils.upload_artifacts = lambda tmpdir: tmpdir
    except Exception:
        pass


def _run(in_maps, trace=False, tmpdir=None):
    from concourse.bass_utils import run_bass_kernel_spmd

    if "nc" not in _CACHE:
        _setup_axon_profile_hook()
        _CACHE["nc"] = _build()
    kw = dict(trace=True, tmpdir=tmpdir) if trace else {}
    return run_bass_kernel_spmd(_CACHE["nc"], in_maps,
                                core_ids=list(range(8)), **kw)


def kernel(**inputs):
    in_maps = _make_in_maps(**inputs)
    res = _run(in_maps)
    return _unshard(res.results)


def kernel_traced(tmpdir, **inputs):
    in_maps = _make_in_maps(**inputs)
    res = _run(in_maps, trace=True, tmpdir=tmpdir)
    return _unshard(res.results), res



# revision 15
# speedup vs baseline: 1.0237x; 1.0029x over previous
"""Trainium2 Bass kernel for nn_BiLSTM_5970004542177.

Model: 2-layer bidirectional LSTM (Keras gate order i,f,g,o), B=128, T=256,
D=U=256, residual on layer 1, merge_mode='ave'.

Device mapping (8 NeuronCores, SPMD single program, no cross-core comm):
  core = (direction, batch quarter): cores 0-3 forward, 4-7 backward
  (backward = time-reversed input, host un-reverses the output).

Each core runs BOTH layers of its chain at B=32 in transposed layout
(partitions = units, free = batch), chunk-interleaved: layer-1 chunk j-1 is
emitted right after layer-0 chunk j, so the two recurrences' serial
dependency chains overlap across engines.  The input projection W^T x + b
is fused into the same PSUM accumulation group as the per-step recurrence
matmuls (bias rides a third K-tile against a constant ones-row).  Layer 1
reads layer 0's h history directly from SBUF and emits
out = 0.5*(h1 + h0); the host adds fw+bw shards and restores (B, T, U).
"""
import sys

if "/opt/trn_rl_repo" not in sys.path:
    sys.path.insert(0, "/opt/trn_rl_repo")

import numpy as np
import ml_dtypes

B = 32            # per-core batch (128 / 4 quarters)
T = 256
D = 256
U = 256
C = 32            # chunk length (steps)
NC = T // C
GS = 4            # steps per PSUM group
NKW = 3           # proj K-tiles (2 data + bias row)
NKR = 2
NM = 8
CB = C * B

_CACHE = {}


class _Unit:
    """Emission helper for one LSTM layer stream; persistent for the whole
    sequence, indexed by global step s in [0, T)."""

    def __init__(self, nc, mybir, pools, tag, W_sb, R_sb, rhs_fn, hist_ap,
                 h_prev0, c_sb):
        self.nc, self.mybir, self.pools = nc, mybir, pools
        self.tag = tag
        self.W_sb, self.R_sb = W_sb, R_sb
        self.rhs_fn, self.hist_ap = rhs_fn, hist_ap
        self.h_prev0, self.c_sb = h_prev0, c_sb
        self.zp = None
        self.zp_next = None

    def _proj_mms(self, zp, g, m_lo, m_hi):
        nc = self.nc
        for m in range(m_lo, m_hi):
            for k in range(NKW):
                nc.tensor.matmul(
                    zp[:, m, :],
                    self.W_sb[:, (m * NKW + k) * 128:(m * NKW + k + 1) * 128],
                    self.rhs_fn(k, g),
                    start=(k == 0 and (m * GS * B) % 512 == 0), stop=False,
                    skip_group_check=True,
                )

    def _new_zp(self):
        F32 = self.mybir.dt.float32
        zp_t = self.pools["psum"].tile([128, NM, GS * B], F32,
                                       tag="zp" + self.tag)
        return zp_t

    def emit_proj_phase(self, g, phase):
        """Emit part of group g's projection: phases 0/1/2 cover m-strips
        0:3 / 3:6 / 6:8.  Phases run in slots g*GS-4 .. g*GS-2 so the last
        strip lands a full step before the group's first recurrence matmul
        needs the PSUM bank."""
        if phase == 0:
            self.zp_next = self._new_zp()
            self._proj_mms(self.zp_next, g, 0, 3)
        elif phase == 1:
            self._proj_mms(self.zp_next, g, 3, 6)
        else:
            self._proj_mms(self.zp_next, g, 6, 8)

    def advance_group(self):
        self.zp = self.zp_next

    def emit_rec_mms(self, s):
        nc = self.nc
        sl = s % GS
        h_prev = self.h_prev0 if s == 0 else self.hist_ap[:, s - 1]
        for m in range(NM):
            for k in range(NKR):
                nc.tensor.matmul(
                    self.zp[:, m, sl * B:(sl + 1) * B],
                    self.R_sb[:, (m * NKR + k) * 128:(m * NKR + k + 1) * 128],
                    h_prev[:, k, :],
                    start=False, stop=(k == NKR - 1),
                    skip_group_check=True,
                )

    def emit_sigmoid_a(self, s):
        """Gates are host-packed in (i, g, f, o) strip order; half A covers
        i and g (strips 0:4) so the t1/t2 chain can start while the m4-7
        recurrence matmuls and sigmoid B are still in flight."""
        nc, mybir = self.nc, self.mybir
        BF16 = mybir.dt.bfloat16
        SIG = mybir.ActivationFunctionType.Sigmoid
        work = self.pools["work"]
        sl = s % GS
        self.gt = work.tile([128, NM, B], BF16, tag="gt" + self.tag)
        zs = self.zp[:, :, sl * B:(sl + 1) * B]
        nc.scalar.activation(self.gt[:, 0:4, :], zs[:, 0:4, :], SIG)

    def emit_sigmoid_b(self, s):
        nc, mybir = self.nc, self.mybir
        SIG = mybir.ActivationFunctionType.Sigmoid
        sl = s % GS
        zs = self.zp[:, :, sl * B:(sl + 1) * B]
        nc.scalar.activation(self.gt[:, 4:8, :], zs[:, 4:8, :], SIG)

    def emit_t1(self):
        nc, mybir = self.nc, self.mybir
        BF16 = mybir.dt.bfloat16
        MULT = mybir.AluOpType.mult
        work = self.pools["work"]
        gt = self.gt
        self.t1 = work.tile([128, 2, B], BF16, tag="t1" + self.tag)
        # c = f*c + i*(2*sg - 1) = f*c + (2*(i*sg) - i); g pre-scaled by 2
        # on the host so tanh(zg) = 2*sigmoid(2 zg) - 1
        nc.vector.scalar_tensor_tensor(self.t1[:], gt[:, 0:2, :], 1.0,
                                       gt[:, 2:4, :], op0=MULT, op1=MULT)

    def emit_t2(self):
        nc, mybir = self.nc, self.mybir
        BF16 = mybir.dt.bfloat16
        MULT = mybir.AluOpType.mult
        SUB = mybir.AluOpType.subtract
        work = self.pools["work"]
        self.t2 = work.tile([128, 2, B], BF16, tag="t2" + self.tag)
        nc.vector.scalar_tensor_tensor(self.t2[:], self.t1[:], 2.0,
                                       self.gt[:, 0:2, :], op0=MULT, op1=SUB)

    def emit_cmul(self):
        nc, mybir = self.nc, self.mybir
        MULT = mybir.AluOpType.mult
        nc.vector.tensor_tensor(self.c_sb[:], self.c_sb[:],
                                self.gt[:, 4:6, :], op=MULT)

    def emit_cadd(self):
        nc, mybir = self.nc, self.mybir
        ADD = mybir.AluOpType.add
        nc.vector.tensor_tensor(self.c_sb[:], self.c_sb[:], self.t2[:],
                                op=ADD)

    def emit_tanh(self):
        nc, mybir = self.nc, self.mybir
        BF16 = mybir.dt.bfloat16
        TANH = mybir.ActivationFunctionType.Tanh
        work = self.pools["work"]
        self.tct = work.tile([128, 2, B], BF16, tag="tc" + self.tag)
        nc.scalar.activation(self.tct[:], self.c_sb[:], TANH)

    def emit_ho(self, s):
        nc, mybir = self.nc, self.mybir
        MULT = mybir.AluOpType.mult
        nc.vector.scalar_tensor_tensor(self.hist_ap[:, s], self.gt[:, 6:8, :],
                                       1.0, self.tct[:], op0=MULT, op1=MULT)


def _build():
    import concourse.bacc as bacc
    import concourse.tile as tile
    from concourse import mybir

    F32 = mybir.dt.float32
    BF16 = mybir.dt.bfloat16
    ADD = mybir.AluOpType.add

    nc = bacc.Bacc("TRN2", target_bir_lowering=False, debug=False)
    W0d = nc.dram_tensor("Wp0", [128, NKW * NM * 128], BF16,
                         kind="ExternalInput")
    R0d = nc.dram_tensor("Rp0", [128, NKR * NM * 128], BF16,
                         kind="ExternalInput")
    W1d = nc.dram_tensor("Wp1", [128, NKW * NM * 128], BF16,
                         kind="ExternalInput")
    R1d = nc.dram_tensor("Rp1", [128, NKR * NM * 128], BF16,
                         kind="ExternalInput")
    Xd = nc.dram_tensor("Xp", [128, 2, T * B], BF16, kind="ExternalInput")
    OutD = nc.dram_tensor("Out", [128, 2, T * 2 * B], BF16,
                          kind="ExternalOutput")

    with tile.TileContext(nc) as tc:
        with (
            tc.tile_pool(name="const", bufs=1) as const,
            tc.tile_pool(name="state", bufs=1) as state,
            tc.tile_pool(name="work", bufs=6) as work,
            tc.tile_pool(name="psum", bufs=2, space="PSUM") as psum,
        ):
            W0 = const.tile([128, NKW * NM * 128], BF16)
            R0 = const.tile([128, NKR * NM * 128], BF16)
            W1 = const.tile([128, NKW * NM * 128], BF16)
            R1 = const.tile([128, NKR * NM * 128], BF16)
            nc.sync.dma_start(out=W0[:], in_=W0d[:])
            nc.sync.dma_start(out=R0[:], in_=R0d[:])
            nc.sync.dma_start(out=W1[:], in_=W1d[:])
            nc.sync.dma_start(out=R1[:], in_=R1d[:])

            xin = const.tile([128, 2, T * B], BF16)
            # per-chunk slices so chunk 0's matmuls start after 1/NC of the
            # input transfer instead of the whole 4 MB
            for jj in range(NC):
                nc.sync.dma_start(out=xin[:, :, jj * CB:(jj + 1) * CB],
                                  in_=Xd[:, :, jj * CB:(jj + 1) * CB])
            ones = const.tile([128, GS * B], BF16)
            nc.vector.memset(ones[:], 0.0)
            nc.vector.memset(ones[0:1, :], 1.0)

            hist0 = state.tile([128, T, 2, B], BF16)
            hist1 = state.tile([128, T, 2, B], BF16)
            h00 = state.tile([128, 2, B], BF16)
            c0 = state.tile([128, 2, B], BF16)
            c1 = state.tile([128, 2, B], BF16)
            nc.vector.memset(h00[:], 0.0)
            nc.vector.memset(c0[:], 0.0)
            nc.vector.memset(c1[:], 0.0)

            pools = {"psum": psum, "work": work}

            def rhs_l0(k, g):
                if k < 2:
                    a = g * GS
                    return xin[:, k, a * B:(a + GS) * B]
                return ones[:]

            def rhs_l1(k, g):
                if k < 2:
                    a = g * GS
                    return hist0[:, a:a + GS, k, :]
                return ones[:]

            NGTOT = T // GS
            u0 = _Unit(nc, mybir, pools, "a", W0, R0, rhs_l0, hist0, h00, c0)
            u1 = _Unit(nc, mybir, pools, "b", W1, R1, rhs_l1, hist1, h00, c1)

            # prologue: group 0 of layer 0 projected up-front
            for p in range(3):
                u0.emit_proj_phase(0, p)

            # u1's (layer-1) tanh/ho for step s are deferred to slot s+1 so
            # the Act queue per slot reads [sig0(s), tanh1(s-1), sig1(s),
            # tanh0(s)].  Projection phases for each group g run in slots
            # g*GS-4 .. g*GS-2, so the last proj matmul never sits between
            # a slot's recurrence bursts at a group boundary.
            pending = None  # (unit, s) whose tanh/ho still to emit
            for t in range(T + C):
                s0 = t if t < T else None
                s1 = t - C if t >= C else None
                if s0 is not None:
                    if s0 % GS == 0:
                        u0.advance_group()
                    u0.emit_rec_mms(s0)
                    u0.emit_sigmoid_a(s0)
                    u0.emit_sigmoid_b(s0)
                    u0.emit_t1()
                    u0.emit_t2()
                    u0.emit_cmul()
                    u0.emit_cadd()
                if pending is not None:
                    pu, ps = pending
                    pu.emit_tanh()
                    pu.emit_ho(ps)
                    pending = None
                if s1 is not None:
                    if s1 % GS == 0:
                        u1.advance_group()
                    u1.emit_rec_mms(s1)
                    u1.emit_sigmoid_a(s1)
                    u1.emit_sigmoid_b(s1)
                    u1.emit_t1()
                    u1.emit_t2()
                    u1.emit_cmul()
                    u1.emit_cadd()
                    pending = (u1, s1)
                if s0 is not None:
                    u0.emit_tanh()
                    u0.emit_ho(s0)
                # projection pre-emission (phase p of group tgt at local
                # step tgt*GS - 4 + p); u1's local step t-C goes negative
                # during the warmup slots, covering its group 0.
                for u, sraw, lo in ((u0, t, 1), (u1, t - C, 0)):
                    tgt, p = (sraw + 4) // GS, (sraw + 4) % GS
                    if p < 3 and lo <= tgt < NGTOT and sraw + 4 >= 0:
                        u.emit_proj_phase(tgt, p)
                # ship both layers' raw h chunks; the host does h0+h1.
                if s0 is not None and s0 % C == C - 1:
                    j = s0 // C
                    nc.sync.dma_start(
                        out=OutD[:, 0, j * C * 2 * B:(j + 1) * C * 2 * B],
                        in_=hist0[:, j * C:(j + 1) * C].rearrange(
                            "p c k b -> p (c k b)"))
                # hist1 chunk k-1 is fully written once the deferred tail
                # flushed at the start of slot where s1 == k*C
                if s1 is not None and s1 % C == 0 and s1 >= C:
                    i = s1 // C - 1
                    nc.sync.dma_start(
                        out=OutD[:, 1, i * C * 2 * B:(i + 1) * C * 2 * B],
                        in_=hist1[:, i * C:(i + 1) * C].rearrange(
                            "p c k b -> p (c k b)"))
            # flush the last pending tail and the final hist1 chunk
            if pending is not None:
                pu, ps = pending
                pu.emit_tanh()
                pu.emit_ho(ps)
            nc.sync.dma_start(
                out=OutD[:, 1, (NC - 1) * C * 2 * B:NC * C * 2 * B],
                in_=hist1[:, (NC - 1) * C:NC * C].rearrange(
                    "p c k b -> p (c k b)"))

    nc.compile()
    return nc


# ------------------------------------------------------------- host packing
def _pack_W_aug(W, b):
    out = np.zeros((128, NKW * NM * 128), np.float32)
    for m in range(NM):
        for k in range(NKW):
            col = (m * NKW + k) * 128
            if k < 2:
                out[:, col:col + 128] = W[k * 128:(k + 1) * 128,
                                          m * 128:(m + 1) * 128]
            else:
                out[0, col:col + 128] = b[m * 128:(m + 1) * 128]
    return out.astype(ml_dtypes.bfloat16)


def _pack_R(R):
    out = np.zeros((128, NKR * NM * 128), np.float32)
    for m in range(NM):
        for k in range(NKR):
            col = (m * NKR + k) * 128
            out[:, col:col + 128] = R[k * 128:(k + 1) * 128,
                                      m * 128:(m + 1) * 128]
    return out.astype(ml_dtypes.bfloat16)


def _pack_x(xs):
    """xs (B, T, D) -> [128, 2, T*B] bf16 (k-tile, t-major cols)."""
    xt = np.ascontiguousarray(np.transpose(xs, (2, 1, 0))).reshape(D, T * B)
    out = np.empty((128, 2, T * B), np.float32)
    out[:, 0, :] = xt[0:128]
    out[:, 1, :] = xt[128:256]
    return out.astype(ml_dtypes.bfloat16)


def _make_in_maps(x, kernels_fw, rec_fw, bias_fw, kernels_bw, rec_bw, bias_bw):
    x = np.asarray(x, np.float32)
    xr = x[:, ::-1, :]
    def g2(a):
        """Pre-scale the g gate by 2 (tanh-via-sigmoid trick), then permute
        gate blocks (i,f,g,o) -> (i,g,f,o) so each sigmoid half covers a
        contiguous strip range on device."""
        a = np.array(a, np.float32)
        a[..., 2 * U:3 * U] *= 2.0
        return np.concatenate([a[..., 0:U], a[..., 2 * U:3 * U],
                               a[..., U:2 * U], a[..., 3 * U:4 * U]], axis=-1)

    packs = {}
    for d, Ws, Rs, bs in (("fw", kernels_fw, rec_fw, bias_fw),
                          ("bw", kernels_bw, rec_bw, bias_bw)):
        packs[d] = [
            (_pack_W_aug(g2(Ws[li]), g2(bs[li])), _pack_R(g2(Rs[li])))
            for li in range(2)
        ]
    in_maps = []
    for core in range(8):
        d = "fw" if core < 4 else "bw"
        q = core % 4
        xs = (x if d == "fw" else xr)[q * B:(q + 1) * B]
        (W0, R0), (W1, R1) = packs[d]
        in_maps.append({"Wp0": W0, "Rp0": R0, "Wp1": W1, "Rp1": R1,
                        "Xp": _pack_x(xs)})
    return in_maps


def _unshard(results):
    full = np.zeros((128, T, U), np.float32)
    for core in range(8):
        d_rev = core >= 4
        q = core % 4
        raw = np.asarray(results[core]["Out"], dtype=np.float32)
        o = (raw[:, 0, :] + raw[:, 1, :]).reshape(128, T, 2, B)
        o = np.transpose(o, (3, 1, 2, 0)).reshape(B, T, U)
        if d_rev:
            o = o[:, ::-1, :]
        full[q * B:(q + 1) * B] += o
    full *= 0.5
    return full


def _setup_axon_profile_hook():
    try:
        import types
        import antenv
        mod = sys.modules.get("antenv.axon_hooks")
        if mod is None:
            mod = types.ModuleType("antenv.axon_hooks")
            holder = {"hook": None}
            mod.set_axon_ntff_profile_hook = lambda h: holder.update(hook=h)
            mod.get_axon_ntff_profile_hook = lambda: holder["hook"]
            sys.modules["antenv.axon_hooks"] = mod
            antenv.axon_hooks = mod
        from trn_agent_boot.trn_boot import _ntff_profile_via_ctypes
        hook = _ntff_profile_via_ctypes("/opt/axon/libaxon_pjrt.so")
        if hook is not None:
            mod.set_axon_ntff_profile_hook(hook)
        import concourse.bass_utils as bass_utils
        bass_utils.upload_artifacts = lambda tmpdir: tmpdir
    except Exception:
        pass


def _run(in_maps, trace=False, tmpdir=None):
    from concourse.bass_utils import run_bass_kernel_spmd

    if "nc" not in _CACHE:
        _setup_axon_profile_hook()
        _CACHE["nc"] = _build()
    kw = dict(trace=True, tmpdir=tmpdir) if trace else {}
    return run_bass_kernel_spmd(_CACHE["nc"], in_maps,
                                core_ids=list(range(8)), **kw)


def kernel(**inputs):
    in_maps = _make_in_maps(**inputs)
    res = _run(in_maps)
    return _unshard(res.results)


def kernel_traced(tmpdir, **inputs):
    in_maps = _make_in_maps(**inputs)
    res = _run(in_maps, trace=True, tmpdir=tmpdir)
    return _unshard(res.results), res



# revision 16
# speedup vs baseline: 1.0511x; 1.0268x over previous
"""Trainium2 Bass kernel for nn_BiLSTM_5970004542177.

Model: 2-layer bidirectional LSTM (Keras gate order i,f,g,o), B=128, T=256,
D=U=256, residual on layer 1, merge_mode='ave'.

Device mapping (8 NeuronCores, SPMD single program, no cross-core comm):
  core = (direction, batch quarter): cores 0-3 forward, 4-7 backward
  (backward = time-reversed input, host un-reverses the output).

Each core runs BOTH layers of its chain at B=32 in transposed layout
(partitions = units, free = batch), chunk-interleaved: layer-1 chunk j-1 is
emitted right after layer-0 chunk j, so the two recurrences' serial
dependency chains overlap across engines.  The input projection W^T x + b
is fused into the same PSUM accumulation group as the per-step recurrence
matmuls (bias rides a third K-tile against a constant ones-row).  Layer 1
reads layer 0's h history directly from SBUF and emits
out = 0.5*(h1 + h0); the host adds fw+bw shards and restores (B, T, U).
"""
import sys

if "/opt/trn_rl_repo" not in sys.path:
    sys.path.insert(0, "/opt/trn_rl_repo")

import numpy as np
import ml_dtypes

B = 32            # per-core batch (128 / 4 quarters)
T = 256
D = 256
U = 256
C = 32            # chunk length (steps)
NC = T // C
GS = 4            # steps per PSUM group
NKW = 3           # proj K-tiles (2 data + bias row)
NKR = 2
NM = 8
CB = C * B

_CACHE = {}


class _Unit:
    """Emission helper for one LSTM layer; supports fine interleaving."""

    def __init__(self, nc, mybir, pools, tag, W_sb, R_sb, rhs_fn, hist_ap,
                 h_prev0, c_sb):
        self.nc, self.mybir, self.pools = nc, mybir, pools
        self.tag = tag
        self.W_sb, self.R_sb = W_sb, R_sb
        self.rhs_fn, self.hist_ap = rhs_fn, hist_ap
        self.h_prev0, self.c_sb = h_prev0, c_sb
        self.zp = None

    def _proj_mms(self, zp, g, m_lo, m_hi):
        nc = self.nc
        for m in range(m_lo, m_hi):
            for k in range(NKW):
                nc.tensor.matmul(
                    zp[:, m, :],
                    self.W_sb[:, (m * NKW + k) * 128:(m * NKW + k + 1) * 128],
                    self.rhs_fn(k, g),
                    start=(k == 0 and (m * GS * B) % 512 == 0), stop=False,
                    skip_group_check=True,
                )

    def _new_zp(self):
        F32 = self.mybir.dt.float32
        zp_t = self.pools["psum"].tile([128, NM, GS * B], F32,
                                       tag="zp" + self.tag)
        return zp_t

    def emit_proj(self, g):
        self.zp = self._new_zp()
        self._proj_mms(self.zp, g, 0, NM)

    def emit_proj_slice(self, g, sl):
        """Emit a quarter of group g's projection (2 M-strips); used to fill
        PE stalls during the previous group's recurrence steps."""
        if sl == 0:
            self.zp_next = self._new_zp()
        self._proj_mms(self.zp_next, g, 2 * sl, 2 * sl + 2)

    def advance_group(self):
        self.zp = self.zp_next

    def emit_step(self, g, sl):
        nc, mybir = self.nc, self.mybir
        F32 = mybir.dt.float32
        BF16 = mybir.dt.bfloat16
        SIG = mybir.ActivationFunctionType.Sigmoid
        TANH = mybir.ActivationFunctionType.Tanh
        MULT = mybir.AluOpType.mult
        ADD = mybir.AluOpType.add
        SUB = mybir.AluOpType.subtract
        work = self.pools["work"]
        s = g * GS + sl
        h_prev = self.h_prev0 if s == 0 else self.hist_ap[:, s - 1]
        for m in range(NM):
            for k in range(NKR):
                nc.tensor.matmul(
                    self.zp[:, m, sl * B:(sl + 1) * B],
                    self.R_sb[:, (m * NKR + k) * 128:(m * NKR + k + 1) * 128],
                    h_prev[:, k, :],
                    start=False, stop=(k == NKR - 1),
                    skip_group_check=True,
                )
        gt = work.tile([128, NM, B], BF16, tag="gt" + self.tag)
        zs = self.zp[:, :, sl * B:(sl + 1) * B]
        # all four gates through one sigmoid; the g columns were pre-scaled
        # by 2 on the host so tanh(zg) = 2*sigmoid(2 zg) - 1 = 2*gt_g - 1
        nc.scalar.activation(gt[:], zs[:], SIG)
        t1 = work.tile([128, 2, B], F32, tag="t1" + self.tag)
        t2 = work.tile([128, 2, B], F32, tag="t2" + self.tag)
        # c = f*c + i*(2*sg - 1) = f*c + (2*(i*sg) - i)
        nc.vector.tensor_tensor(t1[:], gt[:, 0:2, :], gt[:, 4:6, :], op=MULT)
        nc.vector.scalar_tensor_tensor(t2[:], t1[:], 2.0, gt[:, 0:2, :],
                                       op0=MULT, op1=SUB)
        nc.vector.tensor_tensor(self.c_sb[:], self.c_sb[:], gt[:, 2:4, :],
                                op=MULT)
        nc.vector.tensor_tensor(self.c_sb[:], self.c_sb[:], t2[:], op=ADD)
        tct = work.tile([128, 2, B], BF16, tag="tc" + self.tag)
        nc.scalar.activation(tct[:], self.c_sb[:], TANH)
        nc.vector.tensor_tensor(self.hist_ap[:, s], gt[:, 6:8, :], tct[:],
                                op=MULT)


def _build():
    import concourse.bacc as bacc
    import concourse.tile as tile
    from concourse import mybir

    F32 = mybir.dt.float32
    BF16 = mybir.dt.bfloat16
    ADD = mybir.AluOpType.add

    nc = bacc.Bacc("TRN2", target_bir_lowering=False, debug=False)
    W0d = nc.dram_tensor("Wp0", [128, NKW * NM * 128], BF16,
                         kind="ExternalInput")
    R0d = nc.dram_tensor("Rp0", [128, NKR * NM * 128], BF16,
                         kind="ExternalInput")
    W1d = nc.dram_tensor("Wp1", [128, NKW * NM * 128], BF16,
                         kind="ExternalInput")
    R1d = nc.dram_tensor("Rp1", [128, NKR * NM * 128], BF16,
                         kind="ExternalInput")
    Xd = nc.dram_tensor("Xp", [128, 2, T * B], BF16, kind="ExternalInput")
    OutD = nc.dram_tensor("Out", [128, T * 2 * B], F32, kind="ExternalOutput")

    with tile.TileContext(nc) as tc:
        with (
            tc.tile_pool(name="const", bufs=1) as const,
            tc.tile_pool(name="state", bufs=1) as state,
            tc.tile_pool(name="work", bufs=6) as work,
            tc.tile_pool(name="io", bufs=2) as iop,
            tc.tile_pool(name="psum", bufs=2, space="PSUM") as psum,
        ):
            W0 = const.tile([128, NKW * NM * 128], BF16)
            R0 = const.tile([128, NKR * NM * 128], BF16)
            W1 = const.tile([128, NKW * NM * 128], BF16)
            R1 = const.tile([128, NKR * NM * 128], BF16)
            nc.sync.dma_start(out=W0[:], in_=W0d[:])
            nc.sync.dma_start(out=R0[:], in_=R0d[:])
            nc.sync.dma_start(out=W1[:], in_=W1d[:])
            nc.sync.dma_start(out=R1[:], in_=R1d[:])

            xin = const.tile([128, 2, T * B], BF16)
            # per-chunk slices so chunk 0's matmuls start after 1/NC of the
            # input transfer instead of the whole 4 MB
            for jj in range(NC):
                nc.sync.dma_start(out=xin[:, :, jj * CB:(jj + 1) * CB],
                                  in_=Xd[:, :, jj * CB:(jj + 1) * CB])
            ones = const.tile([128, GS * B], BF16)
            nc.vector.memset(ones[:], 0.0)
            nc.vector.memset(ones[0:1, :], 1.0)

            hist0 = state.tile([128, T, 2, B], BF16)
            hist1 = state.tile([128, T, 2, B], BF16)
            h00 = state.tile([128, 2, B], BF16)
            c0 = state.tile([128, 2, B], F32)
            c1 = state.tile([128, 2, B], F32)
            nc.vector.memset(h00[:], 0.0)
            nc.vector.memset(c0[:], 0.0)
            nc.vector.memset(c1[:], 0.0)

            pools = {"psum": psum, "work": work}

            def rhs_l0(j):
                def fn(k, g):
                    if k < 2:
                        a = j * C + g * GS
                        return xin[:, k, a * B:(a + GS) * B]
                    return ones[:]
                return fn

            def rhs_l1(j):
                def fn(k, g):
                    if k < 2:
                        a = j * C + g * GS
                        return hist0[:, a:a + GS, k, :]
                    return ones[:]
                return fn

            NG = C // GS
            for j in range(NC + 1):
                u0 = u1 = None
                if j < NC:
                    u0 = _Unit(nc, mybir, pools, "a", W0, R0, rhs_l0(j),
                               hist0[:, j * C:(j + 1) * C],
                               h00 if j == 0 else hist0[:, j * C - 1], c0)
                if j >= 1:
                    i = j - 1
                    u1 = _Unit(nc, mybir, pools, "b", W1, R1, rhs_l1(i),
                               hist1[:, i * C:(i + 1) * C],
                               h00 if i == 0 else hist1[:, i * C - 1], c1)
                # step-interleaved emission so each unit's matmuls fill the
                # other's recurrence stalls on the PE queue; the next group's
                # projection matmuls are sliced between steps for the same
                # reason (keeps TensorE fed and HAM warm).
                units = [u for u in (u0, u1) if u is not None]
                for g in range(NG):
                    for u in units:
                        if g == 0:
                            u.emit_proj(0)
                        else:
                            u.advance_group()
                    for sl in range(GS):
                        for u in units:
                            u.emit_step(g, sl)
                        if g + 1 < NG:
                            for u in units:
                                u.emit_proj_slice(g + 1, sl)
                if u1 is not None:
                    i = j - 1
                    out_sb = iop.tile([128, C, 2, B], F32, tag="out")
                    nc.vector.tensor_tensor(out_sb[:],
                                            hist1[:, i * C:(i + 1) * C],
                                            hist0[:, i * C:(i + 1) * C],
                                            op=ADD)
                    nc.sync.dma_start(
                        out=OutD[:, i * C * 2 * B:(i + 1) * C * 2 * B],
                        in_=out_sb.rearrange("p c k b -> p (c k b)"))

    nc.compile()
    return nc


# ------------------------------------------------------------- host packing
def _pack_W_aug(W, b):
    out = np.zeros((128, NKW * NM * 128), np.float32)
    for m in range(NM):
        for k in range(NKW):
            col = (m * NKW + k) * 128
            if k < 2:
                out[:, col:col + 128] = W[k * 128:(k + 1) * 128,
                                          m * 128:(m + 1) * 128]
            else:
                out[0, col:col + 128] = b[m * 128:(m + 1) * 128]
    return out.astype(ml_dtypes.bfloat16)


def _pack_R(R):
    out = np.zeros((128, NKR * NM * 128), np.float32)
    for m in range(NM):
        for k in range(NKR):
            col = (m * NKR + k) * 128
            out[:, col:col + 128] = R[k * 128:(k + 1) * 128,
                                      m * 128:(m + 1) * 128]
    return out.astype(ml_dtypes.bfloat16)


def _pack_x(xs):
    """xs (B, T, D) -> [128, 2, T*B] bf16 (k-tile, t-major cols)."""
    xt = np.ascontiguousarray(np.transpose(xs, (2, 1, 0))).reshape(D, T * B)
    out = np.empty((128, 2, T * B), np.float32)
    out[:, 0, :] = xt[0:128]
    out[:, 1, :] = xt[128:256]
    return out.astype(ml_dtypes.bfloat16)


def _make_in_maps(x, kernels_fw, rec_fw, bias_fw, kernels_bw, rec_bw, bias_bw):
    x = np.asarray(x, np.float32)
    xr = x[:, ::-1, :]
    def g2(a):
        a = np.array(a, np.float32)
        a[..., 2 * U:3 * U] *= 2.0
        return a

    packs = {}
    for d, Ws, Rs, bs in (("fw", kernels_fw, rec_fw, bias_fw),
                          ("bw", kernels_bw, rec_bw, bias_bw)):
        packs[d] = [
            (_pack_W_aug(g2(Ws[li]), g2(bs[li])), _pack_R(g2(Rs[li])))
            for li in range(2)
        ]
    in_maps = []
    for core in range(8):
        d = "fw" if core < 4 else "bw"
        q = core % 4
        xs = (x if d == "fw" else xr)[q * B:(q + 1) * B]
        (W0, R0), (W1, R1) = packs[d]
        in_maps.append({"Wp0": W0, "Rp0": R0, "Wp1": W1, "Rp1": R1,
                        "Xp": _pack_x(xs)})
    return in_maps


def _unshard(results):
    full = np.zeros((128, T, U), np.float32)
    for core in range(8):
        d_rev = core >= 4
        q = core % 4
        o = results[core]["Out"].reshape(128, T, 2, B)
        o = np.transpose(o, (3, 1, 2, 0)).reshape(B, T, U)
        if d_rev:
            o = o[:, ::-1, :]
        full[q * B:(q + 1) * B] += o
    full *= 0.5
    return full


def _setup_axon_profile_hook():
    try:
        import types
        import antenv
        mod = sys.modules.get("antenv.axon_hooks")
        if mod is None:
            mod = types.ModuleType("antenv.axon_hooks")
            holder = {"hook": None}
            mod.set_axon_ntff_profile_hook = lambda h: holder.update(hook=h)
            mod.get_axon_ntff_profile_hook = lambda: holder["hook"]
            sys.modules["antenv.axon_hooks"] = mod
            antenv.axon_hooks = mod
        from trn_agent_boot.trn_boot import _ntff_profile_via_ctypes
        hook = _ntff_profile_via_ctypes("/opt/axon/libaxon_pjrt.so")
        if hook is not None:
            mod.set_axon_ntff_profile_hook(hook)
        import concourse.bass_utils as bass_utils
        bass_utils.upload_artifacts = lambda tmpdir: tmpdir
    except Exception:
        pass


def _run(in_maps, trace=False, tmpdir=None):
    from concourse.bass_utils import run_bass_kernel_spmd

    if "nc" not in _CACHE:
        _setup_axon_profile_hook()
        _CACHE["nc"] = _build()
    kw = dict(trace=True, tmpdir=tmpdir) if trace else {}
    return run_bass_kernel_spmd(_CACHE["nc"], in_maps,
                                core_ids=list(range(8)), **kw)


def kernel(**inputs):
    in_maps = _make_in_maps(**inputs)
    res = _run(in_maps)
    return _unshard(res.results)


def kernel_traced(tmpdir, **inputs):
    in_maps = _make_in_maps(**inputs)
    res = _run(in_maps, trace=True, tmpdir=tmpdir)
    return _unshard(res.results), res

